# revision 1
# baseline (speedup 1.0000x reference)
"""Trainium2 Bass kernel for nn_BaselineGAT (LayerNorm + MLP + GATConv).

Strategy (8 NeuronCores, SPMD, host-mediated phase boundary):
  Phase 1 (per core, nodes sharded 6272/core, degree-bucketed order):
    LayerNorm folded into the first matmul (stats via ones-matmul + Square),
    MLP 1488->1024->512 with bf16 matmuls (fp32 PSUM accumulate), then
    row-major heads. Writes per node: a packed g-table row
    [g (256, c-major) | a_src (8)] in bf16 (768B rows), res (256) f32,
    a_dst (8) f32.
  Host: concat g-table shards -> full table [50176, 384] bf16; patch two
    sentinel rows (g=0, a_src=-200) at rows 0 and 32768 (dummy positions);
    padding gather slots point at the sentinel, so no masking is needed
    (exp(lrelu(-200+a_dst)) ~ 4e-18 and g=0).
  Phase 2 (per core, edges sharded by dst, fused): per 128-dst batch,
    gather src rows in <=32-slot chunks from the two table halves (int16
    gather indices limit a half to 32768 rows), e=lrelu(a_src+a_dst),
    exp written into the row, msg = g*ex in place (c-major layout keeps the
    DVE in 2x 16-bit mode), pairwise-tree reduce [g|.|ex] -> per-batch f32
    accumulator, then normalize by the summed ex, +bg, elu, transpose to
    h-major, +res -> final output rows. No separate merge phase.
"""

import sys

sys.path.insert(0, "/opt/trn_rl_repo")

from dataclasses import dataclass

import numpy as np
import ml_dtypes

import concourse.bass as bass  # noqa: F401
import concourse.mybir as mybir
import concourse.tile as tile
from concourse import bacc
from concourse.bass_utils import run_bass_kernel_spmd
from concourse.library_config import mlp as mlp_lib

P = 128
F32 = mybir.dt.float32
BF16 = mybir.dt.bfloat16
I16 = mybir.dt.int16
AL = mybir.AluOpType
AF = mybir.ActivationFunctionType
NP_BF16 = ml_dtypes.bfloat16


@dataclass
class Cfg:
    n_nodes: int = 50000
    n_edges: int = 800000
    d_in: int = 1488
    d_hid: int = 1024
    d_out: int = 512
    C: int = 32
    H: int = 8
    n_cores: int = 8
    node_chunk: int = 512   # phase-1 nodes per chunk
    split_cap: int = 32768  # max rows addressable by int16 gather idx
    slot_chunk: int = 32    # phase-2 gather slots per compute chunk

    @property
    def d_head(self):  # H*C
        return self.C * self.H

    @property
    def d_in_pad(self):
        return ((self.d_in + P - 1) // P) * P

    @property
    def rows_per_core(self):
        nb = (self.n_nodes + P - 1) // P
        nb = ((nb + self.n_cores - 1) // self.n_cores) * self.n_cores
        return nb // self.n_cores * P

    @property
    def n_batches(self):
        return self.rows_per_core // P

    @property
    def table_rows(self):
        return self.rows_per_core * self.n_cores

    @property
    def split(self):
        return min(self.split_cap, self.table_rows)

    @property
    def row_w(self):
        # packed table row in bf16: [g 256 | a_src 8 | ex-slot 8 | pad],
        # 256B-multiple for dma_gather: 384 elems = 768B
        return 384

    @property
    def tree_w(self):
        # reduced width: [g 256 | junk 8 | ex 8]
        return self.d_head + 2 * self.H


CFG = Cfg()

_NC_CACHE = {}


# ----------------------------------------------------------------------------
# Phase 1: LayerNorm + MLP + heads (bf16)
# ----------------------------------------------------------------------------

def build_phase1(cfg: Cfg):
    key = ("p1", cfg.n_nodes, cfg.node_chunk)
    if key in _NC_CACHE:
        return _NC_CACHE[key]
    nc = bacc.Bacc("TRN2", target_bir_lowering=False)
    R = cfg.rows_per_core
    KT1 = cfg.d_in_pad // P          # k-tiles layer 1 (12)
    KT2 = cfg.d_hid // P             # k-tiles layer 2 (8)
    KT3 = cfg.d_out // P             # k-tiles layer 3 (4)
    MT1 = cfg.d_hid // P             # m-tiles layer 1 (8)
    MT2 = cfg.d_out // P             # m-tiles layer 2 (4)
    NCH = cfg.node_chunk
    chunk_sizes = [NCH] * (R // NCH)
    if R % NCH:
        assert R % NCH % P == 0
        chunk_sizes.append(R % NCH)
    W3 = cfg.d_head + cfg.H          # 264
    DH = cfg.d_head

    xT = nc.dram_tensor("xT", [cfg.d_in_pad, R], BF16, kind="ExternalInput")
    W1p = nc.dram_tensor("W1p", [cfg.d_in_pad, cfg.d_hid], BF16, kind="ExternalInput")
    W2 = nc.dram_tensor("W2", [cfg.d_hid, cfg.d_out], BF16, kind="ExternalInput")
    Wgp = nc.dram_tensor("Wgp", [cfg.d_out, W3], BF16, kind="ExternalInput")
    Wrp = nc.dram_tensor("Wrp", [cfg.d_out, W3], BF16, kind="ExternalInput")
    w1s = nc.dram_tensor("w1s", [8, cfg.d_hid], BF16, kind="ExternalInput")
    onep = nc.dram_tensor("onep", [8, P], BF16, kind="ExternalInput")
    ones1 = nc.dram_tensor("ones1", [P, 1], BF16, kind="ExternalInput")
    cvec = nc.dram_tensor("cvec", [P, MT1], F32, kind="ExternalInput")
    b2v = nc.dram_tensor("b2v", [P, MT2], F32, kind="ExternalInput")
    brpad = nc.dram_tensor("brpad", [P, W3], F32, kind="ExternalInput")

    gtab = nc.dram_tensor("gtab", [R, cfg.row_w], BF16, kind="ExternalOutput")
    res = nc.dram_tensor("res", [R, DH], F32, kind="ExternalOutput")
    adst = nc.dram_tensor("adst", [R, cfg.H], F32, kind="ExternalOutput")

    inv_din = 1.0 / cfg.d_in

    with tile.TileContext(nc) as tc:
        with (
            tc.tile_pool(name="wpool", bufs=1) as wp,
            tc.tile_pool(name="xpool", bufs=2) as xp,
            tc.tile_pool(name="sqpool", bufs=2) as sqp,
            tc.tile_pool(name="hpool", bufs=2) as hp,
            tc.tile_pool(name="epool", bufs=3) as ep,
            tc.tile_pool(name="stat", bufs=1) as stp,
            tc.tile_pool(name="ps_y", bufs=2, space="PSUM") as ps_y,
            tc.tile_pool(name="ps_s", bufs=1, space="PSUM") as ps_s,
            tc.tile_pool(name="ps_o", bufs=1, space="PSUM") as ps_o,
        ):
            w1_sb = wp.tile([P, KT1, cfg.d_hid], BF16)
            nc.sync.dma_start(w1_sb[:], W1p.rearrange("(kt p) m -> p kt m", p=P))
            w2_sb = wp.tile([P, KT2, cfg.d_out], BF16)
            nc.sync.dma_start(w2_sb[:], W2.rearrange("(kt p) m -> p kt m", p=P))
            wg_sb = wp.tile([P, KT3, W3], BF16)
            nc.sync.dma_start(wg_sb[:], Wgp.rearrange("(kt p) m -> p kt m", p=P))
            wr_sb = wp.tile([P, KT3, W3], BF16)
            nc.sync.dma_start(wr_sb[:], Wrp.rearrange("(kt p) m -> p kt m", p=P))
            w1s_sb = wp.tile([8, cfg.d_hid], BF16)
            nc.sync.dma_start(w1s_sb[:], w1s[:])
            onep_sb = wp.tile([8, P], BF16)
            nc.sync.dma_start(onep_sb[:], onep[:])
            ones1_sb = wp.tile([P, 1], BF16)
            nc.sync.dma_start(ones1_sb[:], ones1[:])
            cvec_sb = wp.tile([P, MT1], F32)
            nc.sync.dma_start(cvec_sb[:], cvec[:])
            b2_sb = wp.tile([P, MT2], F32)
            nc.sync.dma_start(b2_sb[:], b2v[:])
            brp_sb = wp.tile([P, W3], F32)
            nc.sync.dma_start(brp_sb[:], brpad[:])

            ns = 0
            for NCH in chunk_sizes:
                # ---- load xT chunk [P, KT1, NCH] (bf16)
                xt = xp.tile([P, KT1, NCH], BF16, tag="xt")
                nc.sync.dma_start(
                    xt[:], xT.rearrange("(kt p) n -> p kt n", p=P)[:, :, ns:ns + NCH]
                )
                # ---- stats: S1 = ones^T @ x ; S2 = ones^T @ x^2
                s1_ps = ps_s.tile([1, NCH], F32, tag="s1")
                s2_ps = ps_s.tile([1, NCH], F32, tag="s2")
                for kt in range(KT1):
                    nc.tensor.matmul(s1_ps[:], ones1_sb[:], xt[:, kt],
                                     start=(kt == 0), stop=(kt == KT1 - 1))
                for kt in range(KT1):
                    xsq = sqp.tile([P, NCH], BF16, tag="xsq")
                    nc.scalar.activation(xsq[:], xt[:, kt], AF.Square)
                    nc.tensor.matmul(s2_ps[:], ones1_sb[:], xsq[:],
                                     start=(kt == 0), stop=(kt == KT1 - 1))
                # ---- finalize stats: mu, rstd
                mu_bf = stp.tile([8, NCH], BF16, tag="mu")
                nc.vector.memset(mu_bf[:], 0.0)
                nc.vector.tensor_scalar_mul(mu_bf[0:1, :], s1_ps[:], inv_din)
                mu_f = stp.tile([1, NCH], F32, tag="muf")
                nc.vector.tensor_scalar_mul(mu_f[:], s1_ps[:], inv_din)
                musq = stp.tile([1, NCH], F32, tag="musq")
                nc.vector.tensor_tensor(musq[:], mu_f[:], mu_f[:], op=AL.mult)
                var = stp.tile([1, NCH], F32, tag="var")
                nc.vector.tensor_scalar(var[:], s2_ps[:], inv_din, None, op0=AL.mult)
                nc.vector.tensor_tensor(var[:], var[:], musq[:], op=AL.subtract)
                nc.vector.tensor_scalar_add(var[:], var[:], 1e-5)
                sd = stp.tile([1, NCH], F32, tag="sd")
                nc.scalar.activation(sd[:], var[:], AF.Sqrt)
                rstd = stp.tile([8, NCH], BF16, tag="rstd")
                nc.vector.memset(rstd[:], 0.0)
                with nc.allow_low_precision(
                        reason="rstd broadcast via bf16 matmul; 0.4% scale ok"):
                    nc.vector.reciprocal(rstd[0:1, :], sd[:])
                # broadcast rstd to [P, NCH] via K=8 matmul
                rb_ps = ps_s.tile([P, NCH], F32, tag="rb")
                nc.tensor.matmul(rb_ps[:], onep_sb[:], rstd[:], start=True, stop=True)
                rstd_b = stp.tile([P, NCH], F32, tag="rstdb")
                nc.vector.tensor_copy(rstd_b[:], rb_ps[:])

                # ---- layer 1: y = W1p^T x - w1sum (x) mu ; h = relu(y*rstd + c)
                h_sb = hp.tile([P, MT1, NCH], BF16, tag="h")
                for mt in range(MT1):
                    y_ps = ps_y.tile([P, NCH], F32, tag="y")
                    for kt in range(KT1):
                        nc.tensor.matmul(y_ps[:], w1_sb[:, kt, mt * P:(mt + 1) * P],
                                         xt[:, kt], start=(kt == 0), stop=False)
                    nc.tensor.matmul(y_ps[:], w1s_sb[:, mt * P:(mt + 1) * P], mu_bf[:],
                                     start=False, stop=True)
                    tmp = ep.tile([P, NCH], F32, tag="l1t")
                    nc.vector.tensor_tensor(tmp[:], y_ps[:], rstd_b[:], op=AL.mult)
                    nc.scalar.activation(h_sb[:, mt], tmp[:], AF.Relu,
                                         bias=cvec_sb[:, mt:mt + 1])

                # ---- layer 2: h2 = W2^T h + b2
                h2_sb = hp.tile([P, MT2, NCH], BF16, tag="h2")
                for mt in range(MT2):
                    y2_ps = ps_y.tile([P, NCH], F32, tag="y")
                    for kt in range(KT2):
                        nc.tensor.matmul(y2_ps[:], w2_sb[:, kt, mt * P:(mt + 1) * P],
                                         h_sb[:, kt], start=(kt == 0), stop=(kt == KT2 - 1))
                    nc.scalar.activation(h2_sb[:, mt], y2_ps[:], AF.Identity,
                                         bias=b2_sb[:, mt:mt + 1])

                # ---- layer 3 (row-major): per 128-node subtile
                for nt in range(NCH // P):
                    g_ps = ps_o.tile([P, W3], F32, tag="gps")
                    r_ps = ps_o.tile([P, W3], F32, tag="rps")
                    for kt in range(KT3):
                        nc.tensor.matmul(g_ps[:], h2_sb[:, kt, nt * P:(nt + 1) * P],
                                         wg_sb[:, kt], start=(kt == 0), stop=(kt == KT3 - 1))
                    for kt in range(KT3):
                        nc.tensor.matmul(r_ps[:], h2_sb[:, kt, nt * P:(nt + 1) * P],
                                         wr_sb[:, kt], start=(kt == 0), stop=(kt == KT3 - 1))
                    gt = ep.tile([P, W3], BF16, tag="gt")
                    nc.vector.tensor_copy(gt[:], g_ps[:])
                    rt = ep.tile([P, W3], F32, tag="rt")
                    nc.vector.tensor_tensor(rt[:], r_ps[:], brp_sb[:], op=AL.add)
                    r0 = ns + nt * P
                    nc.sync.dma_start(gtab[r0:r0 + P, :W3], gt[:])
                    nc.sync.dma_start(res[r0:r0 + P, :], rt[:, :DH])
                    nc.sync.dma_start(adst[r0:r0 + P, :], rt[:, DH:W3])
                ns += NCH
    nc.compile()
    _NC_CACHE[key] = nc
    return nc


# ----------------------------------------------------------------------------
# Phase 2: fused edge pass + epilogue
# ----------------------------------------------------------------------------

def build_phase2(cfg: Cfg, Ka: list, Kb: list):
    """Ka/Kb: per-batch slot capacities for the A half (table[:split]) and
    B half (table[split:]). Joint layout per batch: [A slots | B slots]."""
    key = ("p2", cfg.n_nodes, tuple(Ka), tuple(Kb))
    if key in _NC_CACHE:
        return _NC_CACHE[key]
    nc = bacc.Bacc("TRN2", target_bir_lowering=False)
    R = cfg.rows_per_core
    NB = cfg.n_batches
    RW = cfg.row_w
    TW = cfg.tree_w        # 272
    DH = cfg.d_head        # 256
    H = cfg.H
    C = cfg.C
    SC = cfg.slot_chunk    # 32
    assert len(Ka) == NB and len(Kb) == NB
    cols = 8 * (sum(Ka) + sum(Kb))

    gtab = nc.dram_tensor("gtab", [cfg.table_rows, RW], BF16, kind="ExternalInput")
    idx = nc.dram_tensor("idx", [P, cols], I16, kind="ExternalInput")
    adt = nc.dram_tensor("adt", [P, NB, H], BF16, kind="ExternalInput")
    resi = nc.dram_tensor("resi", [R, DH], F32, kind="ExternalInput")
    bgb = nc.dram_tensor("bgb", [P, DH], F32, kind="ExternalInput")
    outp = nc.dram_tensor("outp", [R, DH], F32, kind="ExternalOutput")

    with tile.TileContext(nc) as tc:
        with (
            tc.tile_pool(name="const", bufs=1) as cp,
            tc.tile_pool(name="gath", bufs=3) as gp,
            tc.tile_pool(name="wk", bufs=3) as wk,
            tc.tile_pool(name="accp", bufs=2) as accp,
            tc.tile_pool(name="resp", bufs=2) as rp,
            tc.tile_pool(name="outp_", bufs=2) as op_,
        ):
            nc.gpsimd.load_library(mlp_lib)
            idx_sb = cp.tile([P, cols], I16)
            nc.sync.dma_start(idx_sb[:], idx[:])
            adt_sb = cp.tile([P, NB, H], BF16)
            nc.sync.dma_start(adt_sb[:], adt[:])
            bg_sb = cp.tile([P, DH], F32)
            nc.sync.dma_start(bg_sb[:], bgb[:])

            tabA = gtab[:cfg.split, :]
            tabB = gtab[cfg.split:, :]

            off = 0  # global slot offset into idx
            for b in range(NB):
                acc = accp.tile([P, TW], F32, tag="acc", name=f"acc{b}")
                res_t = rp.tile([P, DH], F32, tag="res", name=f"res{b}")
                nc.sync.dma_start(
                    res_t[:], resi.rearrange("(b p) w -> p b w", p=P)[:, b])
                first = True
                for tab_ap, Kh in ((tabA, Ka[b]), (tabB, Kb[b])):
                    for c0 in range(0, Kh, SC):
                        kc = min(SC, Kh - c0)
                        gt_full = gp.tile([P, SC, RW], BF16, tag="gt",
                                          name=f"g{b}_{off}")
                        gt = gt_full[:, :kc, :]
                        for k0 in range(0, kc, 8):
                            kk = min(8, kc - k0)
                            ni = P * kk
                            nc.gpsimd.dma_gather(
                                gt[:, k0:k0 + kk, :], tab_ap,
                                idx_sb[:, 8 * (off + k0):8 * (off + k0 + kk)],
                                ni, ni, RW,
                            )
                        # e = lrelu(a_src + a_dst); ex = exp(e) -> row slot
                        e_t = wk.tile([P, SC, H], BF16, tag="et")
                        nc.vector.tensor_tensor(
                            e_t[:, :kc], gt[:, :, DH:DH + H],
                            adt_sb[:, b, :].unsqueeze(1).to_broadcast([P, kc, H]),
                            op=AL.add)
                        nc.vector.scalar_tensor_tensor(
                            e_t[:, :kc], e_t[:, :kc], 0.2, e_t[:, :kc],
                            op0=AL.mult, op1=AL.max)
                        nc.scalar.activation(gt[:, :, DH + H:TW], e_t[:, :kc], AF.Exp)
                        # msg = g * ex (broadcast ex over C; c-major keeps 2x)
                        nc.vector.tensor_tensor(
                            gt[:, :, :DH].rearrange("p k (c h) -> p k c h", h=H),
                            gt[:, :, :DH].rearrange("p k (c h) -> p k c h", h=H),
                            gt[:, :, DH + H:TW].unsqueeze(2).to_broadcast(
                                [P, kc, C, H]),
                            op=AL.mult)
                        # pairwise-tree reduce over slots (bf16, packed rows)
                        k = kc
                        while k > 1:
                            hh = (k + 1) // 2
                            lo = k - hh
                            nc.vector.tensor_tensor(
                                gt[:, :lo, :TW], gt[:, :lo, :TW],
                                gt[:, hh:k, :TW], op=AL.add)
                            k = hh
                        if first:
                            nc.vector.tensor_copy(acc[:], gt[:, 0, :TW])
                            first = False
                        else:
                            nc.vector.tensor_tensor(acc[:], acc[:], gt[:, 0, :TW],
                                                    op=AL.add)
                        off += kc
                # ---- epilogue for batch b
                rec = wk.tile([P, H], F32, tag="rec")
                nc.vector.reciprocal(rec[:], acc[:, DH + H:TW])
                nc.vector.tensor_tensor(
                    acc[:, :DH].rearrange("p (c h) -> p c h", h=H),
                    acc[:, :DH].rearrange("p (c h) -> p c h", h=H),
                    rec[:].unsqueeze(1).to_broadcast([P, C, H]),
                    op=AL.mult)
                nc.vector.tensor_tensor(acc[:, :DH], acc[:, :DH], bg_sb[:],
                                        op=AL.add)
                zm = wk.tile([P, DH], F32, tag="zm")
                nc.vector.tensor_scalar_min(zm[:], acc[:, :DH], 0.0)
                ez = wk.tile([P, DH], F32, tag="ez")
                nc.scalar.activation(ez[:], zm[:], AF.Exp)
                o_cm = op_.tile([P, DH], F32, tag="ocm")
                nc.vector.scalar_tensor_tensor(o_cm[:], acc[:, :DH], 0.0, ez[:],
                                               op0=AL.max, op1=AL.add)
                # transpose c-major -> h-major, -1, +res in one op
                o_hm = op_.tile([P, DH], F32, tag="ohm")
                nc.vector.scalar_tensor_tensor(
                    o_hm[:].rearrange("p (h c) -> p h c", c=C),
                    o_cm[:].rearrange("p (c h) -> p c h", h=H).transpose([0, 2, 1]),
                    -1.0,
                    res_t[:].rearrange("p (h c) -> p h c", c=C),
                    op0=AL.add, op1=AL.add)
                nc.sync.dma_start(
                    outp.rearrange("(b p) w -> p b w", p=P)[:, b], o_hm[:])
    nc.compile()
    _NC_CACHE[key] = nc
    return nc


# ----------------------------------------------------------------------------
# Host-side preparation
# ----------------------------------------------------------------------------

def wrap_idx(lst: np.ndarray) -> np.ndarray:
    """list index i -> sbuf [16-wrap x 8 replication]: [p, col] = lst[col*16 + p%16]."""
    n = len(lst)
    assert n % 16 == 0
    lay = lst.reshape(n // 16, 16).T.copy()
    return np.tile(lay, (8, 1)).astype(np.int16)


def prep(cfg: Cfg, x, edge_index, ln_g, ln_b, W1, b1, W2, b2, Wr, br, Wg,
         att_src, att_dst, bg):
    """Everything host-side: sharding, permutations, idx arrays, weight prep."""
    N = cfg.n_nodes
    R = cfg.rows_per_core
    NB = cfg.n_batches
    NCORE = cfg.n_cores
    TR = cfg.table_rows
    H, C = cfg.H, cfg.C

    x = np.asarray(x, np.float32)
    ln_g = np.asarray(ln_g, np.float32)
    ln_b = np.asarray(ln_b, np.float32)
    W1 = np.asarray(W1, np.float32)
    b1 = np.asarray(b1, np.float32)
    W2 = np.asarray(W2, np.float32)
    b2 = np.asarray(b2, np.float32)
    Wr = np.asarray(Wr, np.float32)
    br = np.asarray(br, np.float32)
    Wg = np.asarray(Wg, np.float32)
    att_src = np.asarray(att_src, np.float32)
    att_dst = np.asarray(att_dst, np.float32)
    bg = np.asarray(bg, np.float32)

    src = np.asarray(edge_index[0], np.int64)
    dst = np.asarray(edge_index[1], np.int64)
    loops = np.arange(N, dtype=np.int64)
    src = np.concatenate([src, loops])
    dst = np.concatenate([dst, loops])

    deg = np.bincount(dst, minlength=N)  # in-degree incl self loop

    # ---- provisional node -> position: degree-sorted blocks, round-robin
    order0 = np.argsort(deg, kind="stable")
    padded = np.full(TR, -1, np.int64)
    padded[:N] = order0
    blocks = padded.reshape(TR // P, P)
    core_nodes0 = [[] for _ in range(NCORE)]
    for j in range(blocks.shape[0]):
        core_nodes0[j % NCORE].append(blocks[j])
    core_nodes0 = [np.concatenate(bl) for bl in core_nodes0]

    pos0 = np.full(N, -1, np.int64)
    for c in range(NCORE):
        ids = core_nodes0[c]
        msk = ids >= 0
        pos0[ids[msk]] = c * R + np.nonzero(msk)[0]

    srcA0 = pos0[src] < cfg.split
    degA0 = np.bincount(dst[srcA0], minlength=N)
    degB0 = deg - degA0

    # final assignment: sort (padded) nodes by provisional (degA, degB) so
    # every core's batch b covers the same degA/degB range, re-deal blocks
    keyA = np.where(padded >= 0,
                    degA0[np.maximum(padded, 0)] * 4096
                    + degB0[np.maximum(padded, 0)], -1)
    gorder = np.argsort(keyA, kind="stable")
    sorted_nodes = padded[gorder]

    # force dummies (-1) to global positions 0 and split (sentinel rows).
    # dummies currently sit wherever keyA == -1 sorted them (the front).
    dummy_pos = np.nonzero(sorted_nodes < 0)[0]
    assert len(dummy_pos) >= 2, "need >=2 dummy rows for sentinels"
    # the deal maps sorted-global-index G -> core (G//P) % NCORE,
    # batch (G//P)//NCORE, partition G % P; global table position:
    # core*R + batch*P + partition.
    def table_pos_of_sorted(Gi):
        blk = Gi // P
        return (blk % NCORE) * R + (blk // NCORE) * P + (Gi % P)

    # want table positions 0 and split occupied by dummies: find the sorted
    # indices that land there and swap dummies in.
    targets = [0, cfg.split]
    tp = table_pos_of_sorted(np.arange(TR))
    for t in targets:
        gi = int(np.nonzero(tp == t)[0][0])
        if sorted_nodes[gi] >= 0:
            dj = int(dummy_pos[0]) if sorted_nodes[int(dummy_pos[0])] < 0 else None
            # find a dummy position not already used at a target
            for dcand in dummy_pos:
                gj = int(dcand)
                if sorted_nodes[gj] < 0 and tp[gj] not in targets:
                    sorted_nodes[gi], sorted_nodes[gj] = (
                        sorted_nodes[gj], sorted_nodes[gi])
                    break
            else:
                raise RuntimeError("no free dummy for sentinel swap")

    blocks2 = sorted_nodes.reshape(TR // P, P)
    core_nodes = [[] for _ in range(NCORE)]
    for j in range(blocks2.shape[0]):
        core_nodes[j % NCORE].append(blocks2[j])
    core_nodes = [np.concatenate(bl) for bl in core_nodes]
    pos = np.full(N, -1, np.int64)
    for c in range(NCORE):
        ids = core_nodes[c]
        msk = ids >= 0
        pos[ids[msk]] = c * R + np.nonzero(msk)[0]
    assert core_nodes[0][0] < 0 and core_nodes[cfg.split // R][cfg.split % R] < 0

    # exact halves under final pos
    spos = pos[src]
    dpos = pos[dst]
    isA = spos < cfg.split
    degA = np.zeros(TR, np.int64)
    np.add.at(degA, dpos[isA], 1)
    degB = np.zeros(TR, np.int64)
    np.add.at(degB, dpos[~isA], 1)

    # shared batch capacities (max over cores)
    degA_m = degA.reshape(NCORE, NB, P)
    Ka = np.maximum(1, degA_m.max(axis=(0, 2))).astype(np.int64)
    degB_m = degB.reshape(NCORE, NB, P)
    Kb = np.maximum(1, degB_m.max(axis=(0, 2))).astype(np.int64)

    # ---- per-core edge slot assignment + idx arrays (joint [A|B] layout)
    core = dpos // R
    row = dpos % R
    soff = np.concatenate([[0], np.cumsum(Ka + Kb)])  # slot offset per batch
    nslots = int(soff[-1])
    lists = [np.zeros(nslots * P, np.int64) for _ in range(NCORE)]

    def fill(sel, base_in_batch, base_tab):
        sp = spos[sel] - base_tab
        cr = core[sel]
        rw = row[sel]
        b = rw // P
        p = rw % P
        key = cr * R + rw
        srt = np.argsort(key, kind="stable")
        kk = key[srt]
        grp_start = np.r_[0, np.nonzero(np.diff(kk))[0] + 1]
        sizes = np.diff(np.r_[grp_start, len(kk)])
        within = np.arange(len(kk)) - np.repeat(grp_start, sizes)
        ks = np.zeros(sel.sum(), np.int64)
        ks[srt] = within
        li = (soff[b] + base_in_batch[b] + ks) * P + p
        for c in range(NCORE):
            m = cr == c
            lists[c][li[m]] = sp[m]

    zero_base = np.zeros(NB, np.int64)
    fill(isA, zero_base, 0)
    fill(~isA, Ka, cfg.split)

    idx_w = [wrap_idx(lists[c]) for c in range(NCORE)]

    # ---- phase-1 inputs
    W1p = W1 * ln_g[:, None]
    W1pad = np.zeros((cfg.d_in_pad, cfg.d_hid), np.float32)
    W1pad[:cfg.d_in] = W1p
    w1s = np.zeros((8, cfg.d_hid), np.float32)
    w1s[0] = -W1pad.sum(axis=0)
    cvec_flat = b1 + ln_b @ W1
    cvec = cvec_flat.reshape(cfg.d_hid // P, P).T.astype(np.float32).copy()
    b2t = b2.reshape(cfg.d_out // P, P).T.astype(np.float32).copy()
    onep = np.zeros((8, P), np.float32)
    onep[0] = 1.0
    ones1 = np.ones((P, 1), np.float32)

    att_src_e = np.zeros((cfg.d_head, H), np.float32)
    att_dst_e = np.zeros((cfg.d_head, H), np.float32)
    for h in range(H):
        att_src_e[h * C:(h + 1) * C, h] = att_src[h]
        att_dst_e[h * C:(h + 1) * C, h] = att_dst[h]
    # c-major column permutation for the g table: col c*H+h <- h*C+c
    cm_perm = (np.arange(cfg.d_head).reshape(C, H).T.flatten())  # maps? see below
    # We want Wg_cm[:, c*H + h] = Wg[:, h*C + c]:
    cm_cols = np.empty(cfg.d_head, np.int64)
    for c in range(C):
        for h in range(H):
            cm_cols[c * H + h] = h * C + c
    Wg_cm = Wg[:, cm_cols]
    Wgp = np.concatenate([Wg_cm, Wg @ att_src_e], axis=1).astype(np.float32)
    Wrp = np.concatenate([Wr + 0.0, Wg @ att_dst_e], axis=1).astype(np.float32)

    xts = []
    for c in range(NCORE):
        ids = core_nodes[c]
        xs = np.zeros((R, cfg.d_in), np.float32)
        msk = ids >= 0
        xs[msk] = x[ids[msk]]
        xt = np.zeros((cfg.d_in_pad, R), np.float32)
        xt[:cfg.d_in] = xs.T
        xts.append(xt.astype(NP_BF16))

    bg_cm = bg.reshape(H, C).T.flatten().astype(np.float32)
    bg_b = np.tile(bg_cm, (P, 1))
    W3 = cfg.d_head + cfg.H
    brpad_t = np.zeros((P, W3), np.float32)
    brpad_t[:, :cfg.d_head] = np.tile(br.astype(np.float32), (P, 1))

    meta = dict(core_nodes=core_nodes, pos=pos, Ka=Ka, Kb=Kb,
                idx=idx_w, bg_b=bg_b)
    p1_shared = dict(
        W1p=W1pad.astype(NP_BF16), W2=W2.astype(NP_BF16),
        Wgp=Wgp.astype(NP_BF16), Wrp=Wrp.astype(NP_BF16),
        w1s=w1s.astype(NP_BF16), onep=onep.astype(NP_BF16),
        ones1=ones1.astype(NP_BF16), cvec=cvec, b2v=b2t, brpad=brpad_t)
    p1_maps = [dict(xT=xts[c], **p1_shared) for c in range(NCORE)]
    return p1_maps, meta


def make_sentinel_row(cfg: Cfg) -> np.ndarray:
    row = np.zeros(cfg.row_w, NP_BF16)
    row[cfg.d_head:cfg.d_head + cfg.H] = NP_BF16(-200.0)
    return row


def build_p2_maps(cfg: Cfg, meta, gtabs, ress, adsts):
    gtab_full = np.concatenate(gtabs, axis=0)  # [TR, 384] bf16
    sent = make_sentinel_row(cfg)
    gtab_full[0] = sent
    gtab_full[cfg.split] = sent
    p2_maps = []
    for c in range(cfg.n_cores):
        ad = adsts[c]  # [R, H] f32, π1 order
        adt = ad.reshape(cfg.n_batches, P, cfg.H).transpose(1, 0, 2)
        p2_maps.append(dict(
            gtab=gtab_full, idx=meta["idx"][c],
            adt=adt.astype(NP_BF16).copy(),
            resi=ress[c], bgb=meta["bg_b"],
        ))
    return p2_maps


def kernel(**inputs) -> np.ndarray:
    cfg = CFG
    N = cfg.n_nodes
    NCORE = cfg.n_cores
    DH = cfg.d_head

    p1_maps, meta = prep(cfg, **inputs)

    nc1 = build_phase1(cfg)
    r1 = run_bass_kernel_spmd(nc1, p1_maps, core_ids=list(range(NCORE)))
    gtabs = [r1.results[c]["gtab"] for c in range(NCORE)]
    ress = [r1.results[c]["res"] for c in range(NCORE)]
    adsts = [r1.results[c]["adst"] for c in range(NCORE)]

    Ka = [int(k) for k in meta["Ka"]]
    Kb = [int(k) for k in meta["Kb"]]
    nc2 = build_phase2(cfg, Ka, Kb)
    p2_maps = build_p2_maps(cfg, meta, gtabs, ress, adsts)
    r2 = run_bass_kernel_spmd(nc2, p2_maps, core_ids=list(range(NCORE)))

    out = np.zeros((N, DH), np.float32)
    for c in range(NCORE):
        ids = meta["core_nodes"][c]
        msk = ids >= 0
        out[ids[msk]] = r2.results[c]["outp"][msk]
    return out



# revision 12
# speedup vs baseline: 4.2463x; 4.2463x over previous
"""Trainium2 Bass kernel for nn_BaselineGAT (LayerNorm + MLP + GATConv).

Strategy (8 NeuronCores, SPMD, host-mediated phase boundary):
  Phase 1 (per core, nodes sharded 6272/core, degree-sorted order):
    LayerNorm folded into the first matmul (stats via ones-matmul + Square),
    MLP 1488->1024->512 with bf16 matmuls (fp32 PSUM accumulate), then
    row-major heads (g|a_src and res|a_dst fused into one 528-wide matmul
    chain). Writes per node: a packed g-table row
    [g (256, c-major) | a_src (8)] in bf16 (768B rows), res (256) f32,
    a_dst (8) f32.
  Host: concat g-table shards -> full table [50176, 384] bf16; patch two
    sentinel rows (g=0, a_src=-200) at dummy positions 0 and 3R; padding
    gather slots point at a sentinel, so no masking is needed
    (exp(lrelu(-200+a_dst)) ~ 4e-18 and g=0).
  Phase 2 (per core, edges sharded by dst, fused): nodes are sorted by
    total degree and dealt round-robin so each batch of 128 dst rows x 8
    cores shares tight slot capacities. Edges gather src rows from two
    OVERLAPPING 32768-row table windows A=[0,32768) and B=[17408,50176)
    (int16 gather indices address <=32768 rows); edges whose src falls in
    the overlap are assigned to whichever window balances the per-batch
    (Ka, Kb) capacities (exact per-batch sweep). Per 128-dst batch: gather
    Ka+Kb slots (8-slot/1024-row gather calls; the runtime SWDGE ring is
    fixed at 1024 descriptors -- larger calls crash on HW), one fused
    compute pass: e=lrelu(a_src+a_dst), ex=exp into the row, msg=g*ex in
    place (c-major keeps the DVE in 2x 16-bit mode), pairwise-tree reduce
    [g|.|ex] -> slot 0, then normalize, +bg, elu, transpose to h-major,
    +res -> final output rows.
"""

import sys

sys.path.insert(0, "/opt/trn_rl_repo")

from dataclasses import dataclass

import numpy as np
import ml_dtypes

import concourse.bass as bass  # noqa: F401
import concourse.mybir as mybir
import concourse.tile as tile
from concourse import bacc
from concourse.bass_utils import run_bass_kernel_spmd
from concourse.library_config import mlp as mlp_lib

P = 128
F32 = mybir.dt.float32
BF16 = mybir.dt.bfloat16
I16 = mybir.dt.int16
AL = mybir.AluOpType
AF = mybir.ActivationFunctionType
NP_BF16 = ml_dtypes.bfloat16


@dataclass
class Cfg:
    n_nodes: int = 50000
    n_edges: int = 800000
    d_in: int = 1488
    d_hid: int = 1024
    d_out: int = 512
    C: int = 32
    H: int = 8
    n_cores: int = 8
    node_chunk: int = 512   # phase-1 nodes per chunk
    window: int = 32768     # rows addressable by one int16 gather window
    w2base: int = 8704      # base of the middle gather window
    gather_chunk: int = 8   # phase-2 gather slots per dma_gather call
    ring_bytes: int = 16384  # SWDGE descriptor ring (1024 descs)

    @property
    def d_head(self):  # H*C
        return self.C * self.H

    @property
    def d_in_pad(self):
        return ((self.d_in + P - 1) // P) * P

    @property
    def rows_per_core(self):
        nb = (self.n_nodes + P - 1) // P
        nb = ((nb + self.n_cores - 1) // self.n_cores) * self.n_cores
        return nb // self.n_cores * P

    @property
    def n_batches(self):
        return self.rows_per_core // P

    @property
    def table_rows(self):
        return self.rows_per_core * self.n_cores

    @property
    def baseB(self):
        return self.table_rows - self.window

    @property
    def win_bases(self):
        return (0, self.w2base, self.baseB)

    @property
    def sentB_pos(self):
        # table row used as the w2/w3-window sentinel: core 3, row 0
        return 3 * self.rows_per_core

    @property
    def row_w(self):
        # packed table row in bf16: [g 256 | a_src 8 | ex-slot 8 | pad],
        # 256B-multiple for dma_gather: 384 elems = 768B
        return 384

    @property
    def tree_w(self):
        # reduced width: [g 256 | junk 8 | ex 8]
        return self.d_head + 2 * self.H


CFG = Cfg()

_NC_CACHE = {}


# ----------------------------------------------------------------------------
# Phase 1: LayerNorm + MLP + heads (bf16)
# ----------------------------------------------------------------------------

def build_phase1(cfg: Cfg):
    key = ("p1", cfg.n_nodes, cfg.node_chunk)
    if key in _NC_CACHE:
        return _NC_CACHE[key]
    nc = bacc.Bacc("TRN2", target_bir_lowering=False)
    R = cfg.rows_per_core
    KT1 = cfg.d_in_pad // P          # k-tiles layer 1 (12)
    KT2 = cfg.d_hid // P             # k-tiles layer 2 (8)
    KT3 = cfg.d_out // P             # k-tiles layer 3 (4)
    MT1 = cfg.d_hid // P             # m-tiles layer 1 (8)
    MT2 = cfg.d_out // P             # m-tiles layer 2 (4)
    NCH = cfg.node_chunk
    chunk_sizes = [NCH] * (R // NCH)
    if R % NCH:
        assert R % NCH % P == 0
        chunk_sizes.append(R % NCH)
    W3 = cfg.d_head + cfg.H          # 264
    DH = cfg.d_head

    xT = nc.dram_tensor("xT", [cfg.d_in_pad, R], BF16, kind="ExternalInput")
    W1p = nc.dram_tensor("W1p", [cfg.d_in_pad, cfg.d_hid], BF16, kind="ExternalInput")
    W2 = nc.dram_tensor("W2", [cfg.d_hid, cfg.d_out], BF16, kind="ExternalInput")
    Wgp = nc.dram_tensor("Wgp", [cfg.d_out, W3], BF16, kind="ExternalInput")
    Wrp = nc.dram_tensor("Wrp", [cfg.d_out, W3], BF16, kind="ExternalInput")
    onep = nc.dram_tensor("onep", [8, P], BF16, kind="ExternalInput")
    ones1 = nc.dram_tensor("ones1", [P, 1], BF16, kind="ExternalInput")
    cvec = nc.dram_tensor("cvec", [P, MT1], F32, kind="ExternalInput")
    b2v = nc.dram_tensor("b2v", [P, MT2], F32, kind="ExternalInput")
    brpad = nc.dram_tensor("brpad", [P, W3], F32, kind="ExternalInput")

    gtab = nc.dram_tensor("gtab", [R, cfg.row_w], BF16, kind="ExternalOutput")
    res = nc.dram_tensor("res", [R, DH], F32, kind="ExternalOutput")
    adst = nc.dram_tensor("adst", [R, cfg.H], F32, kind="ExternalOutput")

    inv_din = 1.0 / cfg.d_in

    with tile.TileContext(nc) as tc:
        with (
            tc.tile_pool(name="wpool", bufs=1) as wp,
            tc.tile_pool(name="xpool", bufs=2) as xp,
            tc.tile_pool(name="sqpool", bufs=2) as sqp,
            tc.tile_pool(name="hpool", bufs=2) as hp,
            tc.tile_pool(name="epool", bufs=3) as ep,
            tc.tile_pool(name="stat", bufs=1) as stp,
            tc.tile_pool(name="ps_y", bufs=2, space="PSUM") as ps_y,
            tc.tile_pool(name="ps_s", bufs=1, space="PSUM") as ps_s,
            tc.tile_pool(name="ps_o", bufs=1, space="PSUM") as ps_o,
        ):
            w1_sb = wp.tile([P, KT1, cfg.d_hid], BF16)
            nc.sync.dma_start(w1_sb[:], W1p.rearrange("(kt p) m -> p kt m", p=P))
            w2_sb = wp.tile([P, KT2, cfg.d_out], BF16)
            nc.sync.dma_start(w2_sb[:], W2.rearrange("(kt p) m -> p kt m", p=P))
            wg_sb = wp.tile([P, KT3, W3], BF16)
            nc.sync.dma_start(wg_sb[:], Wgp.rearrange("(kt p) m -> p kt m", p=P))
            wr_sb = wp.tile([P, KT3, W3], BF16)
            nc.sync.dma_start(wr_sb[:], Wrp.rearrange("(kt p) m -> p kt m", p=P))
            onep_sb = wp.tile([8, P], BF16)
            nc.sync.dma_start(onep_sb[:], onep[:])
            ones1_sb = wp.tile([P, 1], BF16)
            nc.sync.dma_start(ones1_sb[:], ones1[:])
            cvec_sb = wp.tile([P, MT1], F32)
            nc.sync.dma_start(cvec_sb[:], cvec[:])
            b2_sb = wp.tile([P, MT2], F32)
            nc.sync.dma_start(b2_sb[:], b2v[:])
            brp_sb = wp.tile([P, W3], F32)
            nc.sync.dma_start(brp_sb[:], brpad[:])

            def stats_part(ns, NCH):
                # ---- load xT chunk [P, KT1, NCH] (bf16)
                xt = xp.tile([P, KT1, NCH], BF16, tag="xt", name=f"xt{ns}")
                nc.sync.dma_start(
                    xt[:], xT.rearrange("(kt p) n -> p kt n", p=P)[:, :, ns:ns + NCH]
                )
                # ---- stats: per-partition kt-tree sums on DVE, then one
                # ones-matmul each for the 128-partition reduction
                xsum = sqp.tile([P, NCH], BF16, tag="xsum")
                xsq = sqp.tile([P, KT1, NCH], BF16, tag="xsq")
                with nc.allow_low_precision(reason="bf16 kt-tree stats; <1e-3"):
                    nc.vector.tensor_tensor(xsum[:], xt[:, 0], xt[:, 1], op=AL.add)
                    for kt in range(2, KT1):
                        nc.vector.tensor_tensor(xsum[:], xsum[:], xt[:, kt],
                                                op=AL.add)
                for kt in range(KT1):
                    nc.scalar.activation(xsq[:, kt], xt[:, kt], AF.Square)
                with nc.allow_low_precision(reason="bf16 kt-tree stats; <1e-3"):
                    k = KT1
                    while k > 1:
                        hh = (k + 1) // 2
                        lo = k - hh
                        nc.vector.tensor_tensor(xsq[:, :lo], xsq[:, :lo],
                                                xsq[:, hh:k], op=AL.add)
                        k = hh
                s1_ps = ps_s.tile([1, NCH], F32, tag="s1")
                s2_ps = ps_s.tile([1, NCH], F32, tag="s2")
                nc.tensor.matmul(s1_ps[:], ones1_sb[:], xsum[:], start=True,
                                 stop=True)
                nc.tensor.matmul(s2_ps[:], ones1_sb[:], xsq[:, 0], start=True,
                                 stop=True)
                # ---- finalize stats: mu, rstd
                mu_bf = stp.tile([8, NCH], BF16, tag="mu")
                nc.vector.memset(mu_bf[:], 0.0)
                nc.vector.tensor_scalar_mul(mu_bf[0:1, :], s1_ps[:], inv_din)
                mu_f = stp.tile([1, NCH], F32, tag="muf")
                nc.vector.tensor_scalar_mul(mu_f[:], s1_ps[:], inv_din)
                musq = stp.tile([1, NCH], F32, tag="musq")
                nc.vector.tensor_tensor(musq[:], mu_f[:], mu_f[:], op=AL.mult)
                var = stp.tile([1, NCH], F32, tag="var")
                nc.vector.tensor_scalar(var[:], s2_ps[:], inv_din, None, op0=AL.mult)
                nc.vector.tensor_tensor(var[:], var[:], musq[:], op=AL.subtract)
                nc.vector.tensor_scalar_add(var[:], var[:], 1e-5)
                sd = stp.tile([1, NCH], F32, tag="sd")
                nc.scalar.activation(sd[:], var[:], AF.Sqrt)
                rstd = stp.tile([8, NCH], BF16, tag="rstd")
                nc.vector.memset(rstd[:], 0.0)
                with nc.allow_low_precision(
                        reason="rstd broadcast via bf16 matmul; 0.4% scale ok"):
                    nc.vector.reciprocal(rstd[0:1, :], sd[:])
                # broadcast mu, rstd to [P, NCH] via K=8 matmuls; center+scale
                # x in place on DVE (removes the per-mt mu matmul + y*rstd)
                rb_ps = ps_s.tile([P, NCH], F32, tag="rb")
                nc.tensor.matmul(rb_ps[:], onep_sb[:], rstd[:], start=True, stop=True)
                rstd_b = stp.tile([P, NCH], BF16, tag="rstdb")
                nc.vector.tensor_copy(rstd_b[:], rb_ps[:])
                mb_ps = ps_s.tile([P, NCH], F32, tag="mb")
                nc.tensor.matmul(mb_ps[:], onep_sb[:], mu_bf[:], start=True, stop=True)
                mu_b = stp.tile([P, NCH], BF16, tag="mub")
                nc.vector.tensor_copy(mu_b[:], mb_ps[:])
                with nc.allow_low_precision(reason="bf16 x centering; ~0.2%"):
                    nc.vector.tensor_tensor(
                        xt[:], xt[:],
                        mu_b[:].unsqueeze(1).to_broadcast([P, KT1, NCH]),
                        op=AL.subtract)
                    nc.vector.tensor_tensor(
                        xt[:], xt[:],
                        rstd_b[:].unsqueeze(1).to_broadcast([P, KT1, NCH]),
                        op=AL.mult)
                return xt

            def mlp_part(xt, ns, NCH):
                # ---- layer 1: h = relu(W1p^T xn + c)
                h_sb = hp.tile([P, MT1, NCH], BF16, tag="h")
                for mt in range(MT1):
                    y_ps = ps_y.tile([P, NCH], F32, tag="y")
                    for kt in range(KT1):
                        nc.tensor.matmul(y_ps[:], w1_sb[:, kt, mt * P:(mt + 1) * P],
                                         xt[:, kt], start=(kt == 0), stop=(kt == KT1 - 1))
                    nc.scalar.activation(h_sb[:, mt], y_ps[:], AF.Relu,
                                         bias=cvec_sb[:, mt:mt + 1])

                # ---- layer 2: h2 = W2^T h + b2
                h2_sb = hp.tile([P, MT2, NCH], BF16, tag="h2")
                for mt in range(MT2):
                    y2_ps = ps_y.tile([P, NCH], F32, tag="y")
                    for kt in range(KT2):
                        nc.tensor.matmul(y2_ps[:], w2_sb[:, kt, mt * P:(mt + 1) * P],
                                         h_sb[:, kt], start=(kt == 0), stop=(kt == KT2 - 1))
                    nc.scalar.activation(h2_sb[:, mt], y2_ps[:], AF.Identity,
                                         bias=b2_sb[:, mt:mt + 1])

                # ---- layer 3 (row-major): per 128-node subtile
                for nt in range(NCH // P):
                    g_ps = ps_o.tile([P, W3], F32, tag="gps")
                    r_ps = ps_o.tile([P, W3], F32, tag="rps")
                    for kt in range(KT3):
                        nc.tensor.matmul(g_ps[:], h2_sb[:, kt, nt * P:(nt + 1) * P],
                                         wg_sb[:, kt], start=(kt == 0), stop=(kt == KT3 - 1))
                    for kt in range(KT3):
                        nc.tensor.matmul(r_ps[:], h2_sb[:, kt, nt * P:(nt + 1) * P],
                                         wr_sb[:, kt], start=(kt == 0), stop=(kt == KT3 - 1))
                    gt = ep.tile([P, W3], BF16, tag="gt")
                    nc.vector.tensor_copy(gt[:], g_ps[:])
                    rt = ep.tile([P, W3], F32, tag="rt")
                    nc.vector.tensor_tensor(rt[:], r_ps[:], brp_sb[:], op=AL.add)
                    r0 = ns + nt * P
                    nc.sync.dma_start(gtab[r0:r0 + P, :W3], gt[:])
                    nc.sync.dma_start(res[r0:r0 + P, :], rt[:, :DH])
                    nc.sync.dma_start(adst[r0:r0 + P, :], rt[:, DH:W3])

            # software-pipelined emission: stats(k+1) lands before mlp(k) so
            # the PE never waits on the centering chain
            ns = 0
            pend = None  # (xt, ns, NCH)
            for NCH in chunk_sizes:
                xt = stats_part(ns, NCH)
                if pend is not None:
                    mlp_part(*pend)
                pend = (xt, ns, NCH)
                ns += NCH
            mlp_part(*pend)
    nc.compile()
    _NC_CACHE[key] = nc
    return nc


# ----------------------------------------------------------------------------
# Phase 2: fused edge pass + epilogue
# ----------------------------------------------------------------------------

def build_phase2(cfg: Cfg, Ks: list):
    """Ks[w][b]: per-batch slot capacities for the three overlapping gather
    windows (bases 0/8704/17408, width 32768). Joint layout per batch:
    [w1 | w2 | w3] slots, one fused compute pass over all of them."""
    key = ("p2", cfg.n_nodes, tuple(map(tuple, Ks)))
    if key in _NC_CACHE:
        return _NC_CACHE[key]
    nc = bacc.Bacc("TRN2", target_bir_lowering=False,
                   dynamic_dma_scratch_size=cfg.ring_bytes)
    R = cfg.rows_per_core
    NB = cfg.n_batches
    RW = cfg.row_w
    TW = cfg.tree_w        # 272
    DH = cfg.d_head        # 256
    H = cfg.H
    C = cfg.C
    GC = cfg.gather_chunk  # 16
    K1, K2, K3 = Ks
    assert len(K1) == NB and len(K2) == NB and len(K3) == NB
    SCtot = [a + b + c for a, b, c in zip(K1, K2, K3)]
    SCmax = max(SCtot)
    cols = 8 * sum(SCtot)

    gtab = nc.dram_tensor("gtab", [cfg.table_rows, RW], BF16, kind="ExternalInput")
    idx = nc.dram_tensor("idx", [P, cols], I16, kind="ExternalInput")
    adt = nc.dram_tensor("adt", [P, NB, H], BF16, kind="ExternalInput")
    resi = nc.dram_tensor("resi", [R, DH], F32, kind="ExternalInput")
    bgb = nc.dram_tensor("bgb", [P, DH], BF16, kind="ExternalInput")
    outp = nc.dram_tensor("outp", [R, DH], F32, kind="ExternalOutput")

    with tile.TileContext(nc) as tc:
        with (
            tc.tile_pool(name="const", bufs=1) as cp,
            tc.tile_pool(name="gath", bufs=3) as gp,
            tc.tile_pool(name="wk", bufs=3) as wk,
            tc.tile_pool(name="resp", bufs=2) as rp,
            tc.tile_pool(name="outp_", bufs=2) as op_,
        ):
            nc.gpsimd.load_library(mlp_lib)
            idx_sb = cp.tile([P, cols], I16)
            nc.sync.dma_start(idx_sb[:], idx[:])
            adt_sb = cp.tile([P, NB, H], BF16)
            nc.sync.dma_start(adt_sb[:], adt[:])
            bg_sb = cp.tile([P, DH], BF16)
            nc.sync.dma_start(bg_sb[:], bgb[:])

            tabs = [gtab[w0:w0 + cfg.window, :] for w0 in cfg.win_bases]

            off = 0  # global slot offset into idx
            for b in range(NB):
                SCb = SCtot[b]
                gt_full = gp.tile([P, SCmax, RW], BF16, tag="gt", name=f"g{b}")
                gt = gt_full[:, :SCb, :]
                res_t = rp.tile([P, DH], F32, tag="res", name=f"res{b}")
                nc.sync.dma_start(
                    res_t[:], resi.rearrange("(b p) w -> p b w", p=P)[:, b])
                for tab_ap, s0, Kh in ((tabs[0], 0, K1[b]),
                                       (tabs[1], K1[b], K2[b]),
                                       (tabs[2], K1[b] + K2[b], K3[b])):
                    for k0 in range(0, Kh, GC):
                        kk = min(GC, Kh - k0)
                        ni = P * kk
                        nc.gpsimd.dma_gather(
                            gt[:, s0 + k0:s0 + k0 + kk, :], tab_ap,
                            idx_sb[:, 8 * (off + k0):8 * (off + k0 + kk)],
                            ni, ni, RW,
                        )
                    off += Kh
                # e = lrelu(a_src + a_dst); ex = exp(e) -> row slot
                e_t = wk.tile([P, SCmax, H], BF16, tag="et")
                nc.vector.tensor_tensor(
                    e_t[:, :SCb], gt[:, :, DH:DH + H],
                    adt_sb[:, b, :].unsqueeze(1).to_broadcast([P, SCb, H]),
                    op=AL.add)
                nc.vector.scalar_tensor_tensor(
                    e_t[:, :SCb], e_t[:, :SCb], 0.2, e_t[:, :SCb],
                    op0=AL.mult, op1=AL.max)
                nc.scalar.activation(gt[:, :, DH + H:TW], e_t[:, :SCb], AF.Exp)
                # msg = g * ex (broadcast ex over C; c-major layout keeps 2x)
                nc.vector.tensor_tensor(
                    gt[:, :, :DH].rearrange("p k (c h) -> p k c h", h=H),
                    gt[:, :, :DH].rearrange("p k (c h) -> p k c h", h=H),
                    gt[:, :, DH + H:TW].unsqueeze(2).to_broadcast(
                        [P, SCb, C, H]),
                    op=AL.mult)
                # pairwise-tree reduce over slots (bf16, packed rows) -> slot 0
                k = SCb
                while k > 1:
                    hh = (k + 1) // 2
                    lo = k - hh
                    nc.vector.tensor_tensor(
                        gt[:, :lo, :TW], gt[:, :lo, :TW],
                        gt[:, hh:k, :TW], op=AL.add)
                    k = hh
                # ---- epilogue for batch b (from gt[:, 0, :TW])
                acc = gt_full[:, 0, :]
                rec = wk.tile([P, H], BF16, tag="rec")
                with nc.allow_low_precision(
                        reason="bf16 alpha-denominator; ~0.4% head scale"):
                    nc.vector.reciprocal(rec[:], acc[:, DH + H:TW])
                    o_cm = op_.tile([P, DH], BF16, tag="ocm")
                    nc.vector.tensor_tensor(
                        o_cm[:].rearrange("p (c h) -> p c h", h=H),
                        acc[:, :DH].rearrange("p (c h) -> p c h", h=H),
                        rec[:].unsqueeze(1).to_broadcast([P, C, H]),
                        op=AL.mult)
                    nc.vector.tensor_tensor(o_cm[:], o_cm[:], bg_sb[:], op=AL.add)
                    zm = wk.tile([P, DH], BF16, tag="zm")
                    nc.vector.tensor_scalar_min(zm[:], o_cm[:], 0.0)
                ez = wk.tile([P, DH], F32, tag="ez")
                nc.scalar.activation(ez[:], zm[:], AF.Exp)
                o_p = op_.tile([P, DH], F32, tag="op")
                nc.vector.scalar_tensor_tensor(o_p[:], o_cm[:], 0.0, ez[:],
                                               op0=AL.max, op1=AL.add)
                # transpose c-major -> h-major, -1, +res in one op
                o_hm = op_.tile([P, DH], F32, tag="ohm")
                nc.vector.scalar_tensor_tensor(
                    o_hm[:].rearrange("p (h c) -> p h c", c=C),
                    o_p[:].rearrange("p (c h) -> p c h", h=H).transpose([0, 2, 1]),
                    -1.0,
                    res_t[:].rearrange("p (h c) -> p h c", c=C),
                    op0=AL.add, op1=AL.add)
                nc.sync.dma_start(
                    outp.rearrange("(b p) w -> p b w", p=P)[:, b], o_hm[:])
    nc.compile()
    _NC_CACHE[key] = nc
    return nc


# ----------------------------------------------------------------------------
# Host-side preparation
# ----------------------------------------------------------------------------

def wrap_idx(lst: np.ndarray) -> np.ndarray:
    """list index i -> sbuf [16-wrap x 8 replication]: [p, col] = lst[col*16 + p%16]."""
    n = len(lst)
    assert n % 16 == 0
    lay = lst.reshape(n // 16, 16).T.copy()
    return np.tile(lay, (8, 1)).astype(np.int16)


def prep(cfg: Cfg, x, edge_index, ln_g, ln_b, W1, b1, W2, b2, Wr, br, Wg,
         att_src, att_dst, bg):
    """Everything host-side: sharding, permutations, idx arrays, weight prep."""
    N = cfg.n_nodes
    R = cfg.rows_per_core
    NB = cfg.n_batches
    NCORE = cfg.n_cores
    TR = cfg.table_rows
    H, C = cfg.H, cfg.C
    W = cfg.window
    baseB = cfg.baseB

    x = np.asarray(x, np.float32)
    ln_g = np.asarray(ln_g, np.float32)
    ln_b = np.asarray(ln_b, np.float32)
    W1 = np.asarray(W1, np.float32)
    b1 = np.asarray(b1, np.float32)
    W2 = np.asarray(W2, np.float32)
    b2 = np.asarray(b2, np.float32)
    Wr = np.asarray(Wr, np.float32)
    br = np.asarray(br, np.float32)
    Wg = np.asarray(Wg, np.float32)
    att_src = np.asarray(att_src, np.float32)
    att_dst = np.asarray(att_dst, np.float32)
    bg = np.asarray(bg, np.float32)

    src = np.asarray(edge_index[0], np.int64)
    dst = np.asarray(edge_index[1], np.int64)
    loops = np.arange(N, dtype=np.int64)
    src = np.concatenate([src, loops])
    dst = np.concatenate([dst, loops])

    deg = np.bincount(dst, minlength=N)  # in-degree incl self loop

    # ---- order: total-degree sort (dummies first), deal blocks round-robin
    keys = np.concatenate([deg, np.full(TR - N, -1, np.int64)])
    nodes = np.concatenate([np.arange(N), np.full(TR - N, -1, np.int64)])
    order = np.argsort(keys, kind="stable")
    sorted_nodes = nodes[order]
    # dummies occupy the lowest sorted positions. Sentinel A lives at table
    # position 0 (= sorted position 0, a dummy). Sentinel B needs a dummy at
    # table position 3R (core 3, row 0) = sorted position 384 (block 3,
    # partition 0): swap a dummy there.
    assert sorted_nodes[0] < 0 and sorted_nodes[1] < 0
    sorted_nodes[1], sorted_nodes[384] = sorted_nodes[384], sorted_nodes[1]

    blocks = sorted_nodes.reshape(TR // P, P)
    core_nodes = [[] for _ in range(NCORE)]
    for j in range(blocks.shape[0]):
        core_nodes[j % NCORE].append(blocks[j])
    core_nodes = [np.concatenate(bl) for bl in core_nodes]
    pos = np.full(N, -1, np.int64)
    for c in range(NCORE):
        ids = core_nodes[c]
        msk = ids >= 0
        pos[ids[msk]] = c * R + np.nonzero(msk)[0]
    assert core_nodes[0][0] < 0 and core_nodes[3][0] < 0
    sentB_local = cfg.sentB_pos - baseB

    # ---- window membership (3 overlapping windows) and per-batch caps
    spos = pos[src]
    dpos = pos[dst]
    w2b = cfg.w2base                  # 8704
    w2e = w2b + W                     # 41472
    w3b = baseB                       # 17408
    # edge class by src row: 0:{1} 1:{1,2} 2:{1,2,3} 3:{2,3} 4:{3}
    cls = np.full(len(src), 2, np.int64)
    cls[spos < w2b] = 0
    cls[(spos >= w2b) & (spos < w3b)] = 1
    cls[(spos >= W) & (spos < w2e)] = 3
    cls[spos >= w2e] = 4

    ccnt = np.zeros((5, TR), np.int64)
    np.add.at(ccnt, (cls, dpos), 1)
    degp = ccnt.sum(0)

    def batch_max(a):
        return a.reshape(NCORE, NB, P).transpose(1, 0, 2).reshape(NB, -1).max(1)

    M1 = batch_max(ccnt[0])
    M3 = batch_max(ccnt[4])
    M12 = batch_max(ccnt[0] + ccnt[1])
    M23 = batch_max(ccnt[3] + ccnt[4])
    M13 = batch_max(ccnt[0] + ccnt[4])
    M = batch_max(degp)

    K1 = np.zeros(NB, np.int64)
    K2 = np.zeros(NB, np.int64)
    K3 = np.zeros(NB, np.int64)
    for b in range(NB):
        best = None
        for k1 in range(int(M1[b]), int(M[b]) + 1):
            for k2 in range(0, int(M[b]) + 1):
                if k1 + k2 < int(M12[b]):
                    continue
                k3 = max(int(M3[b]), int(M23[b]) - k2, int(M13[b]) - k1,
                         int(M[b]) - k1 - k2, 0)
                if best is None or k1 + k2 + k3 < best[0]:
                    best = (k1 + k2 + k3, k1, k2, k3)
        if best is None or best[0] == 0:
            best = (1, 1, 0, 0)
        K1[b], K2[b], K3[b] = best[1], best[2], best[3]

    # ---- per-dst greedy window fill (feasible by the Hall constraints)
    b_of_pos = (np.arange(TR) % R) // P
    K1p, K2p, K3p = K1[b_of_pos], K2[b_of_pos], K3[b_of_pos]
    n1, c12, c123, c23, n3 = ccnt
    x12_1 = np.minimum(c12, K1p - n1)
    x12_2 = c12 - x12_1
    x23_3 = np.minimum(c23, K3p - n3)
    x23_2 = c23 - x23_3
    rem1 = K1p - n1 - x12_1
    rem2 = K2p - x12_2 - x23_2
    rem3 = K3p - n3 - x23_3
    assert (x12_2 >= 0).all() and (x23_2 >= 0).all() and (rem2 >= 0).all()
    y1 = np.minimum(c123, rem1)
    y2 = np.minimum(c123 - y1, rem2)
    y3 = c123 - y1 - y2
    assert (y3 <= rem3).all(), "greedy window fill infeasible"

    # per-edge window: rank within (dst, class), then threshold
    key_c = dpos * 5 + cls
    srt = np.argsort(key_c, kind="stable")
    kk_ = key_c[srt]
    grp_start = np.r_[0, np.nonzero(np.diff(kk_))[0] + 1]
    sizes = np.diff(np.r_[grp_start, len(kk_)])
    within = np.arange(len(kk_)) - np.repeat(grp_start, sizes)
    crank = np.zeros(len(src), np.int64)
    crank[srt] = within

    win = np.zeros(len(src), np.int64)
    win[cls == 0] = 0
    win[cls == 4] = 2
    m = cls == 1
    win[m] = np.where(crank[m] < x12_1[dpos[m]], 0, 1)
    m = cls == 3
    win[m] = np.where(crank[m] < x23_3[dpos[m]], 2, 1)
    m = cls == 2
    win[m] = np.where(
        crank[m] < y1[dpos[m]], 0,
        np.where(crank[m] < (y1 + y2)[dpos[m]], 1, 2))

    li = spos - np.array(cfg.win_bases)[win]
    assert li.min() >= 0 and li.max() < W

    # ---- slot assignment within (dst, window) + idx lists
    core = dpos // R
    row = dpos % R
    SCtot = K1 + K2 + K3
    soff = np.concatenate([[0], np.cumsum(SCtot)])
    nslots = int(soff[-1])
    # default padding: w1 slots -> sentinel A (0), w2/w3 -> shared sentinel
    sent2 = cfg.sentB_pos - w2b
    sent3 = cfg.sentB_pos - w3b
    base_list = np.zeros(nslots * P, np.int64)
    for b in range(NB):
        s2 = (soff[b] + K1[b]) * P
        s3 = (soff[b] + K1[b] + K2[b]) * P
        sE = soff[b + 1] * P
        base_list[s2:s3] = sent2
        base_list[s3:sE] = sent3
    lists = [base_list.copy() for _ in range(NCORE)]

    key2 = (core * R + row) * 3 + win
    srt = np.argsort(key2, kind="stable")
    kk_ = key2[srt]
    grp_start = np.r_[0, np.nonzero(np.diff(kk_))[0] + 1]
    sizes = np.diff(np.r_[grp_start, len(kk_)])
    within = np.arange(len(kk_)) - np.repeat(grp_start, sizes)
    ks = np.zeros(len(src), np.int64)
    ks[srt] = within
    b_of = row // P
    p_of = row % P
    slot = ks + np.where(win == 0, 0,
                         np.where(win == 1, K1[b_of], (K1 + K2)[b_of]))
    lpos = (soff[b_of] + slot) * P + p_of
    for c in range(NCORE):
        m = core == c
        lists[c][lpos[m]] = li[m]

    idx_w = [wrap_idx(lists[c]) for c in range(NCORE)]

    # ---- phase-1 inputs
    W1p = W1 * ln_g[:, None]
    W1pad = np.zeros((cfg.d_in_pad, cfg.d_hid), np.float32)
    W1pad[:cfg.d_in] = W1p
    cvec_flat = b1 + ln_b @ W1
    cvec = cvec_flat.reshape(cfg.d_hid // P, P).T.astype(np.float32).copy()
    b2t = b2.reshape(cfg.d_out // P, P).T.astype(np.float32).copy()
    onep = np.zeros((8, P), np.float32)
    onep[0] = 1.0
    ones1 = np.ones((P, 1), np.float32)

    att_src_e = np.zeros((cfg.d_head, H), np.float32)
    att_dst_e = np.zeros((cfg.d_head, H), np.float32)
    for h in range(H):
        att_src_e[h * C:(h + 1) * C, h] = att_src[h]
        att_dst_e[h * C:(h + 1) * C, h] = att_dst[h]
    # c-major column permutation for the g table: col c*H+h <- h*C+c
    cm_cols = np.empty(cfg.d_head, np.int64)
    for c in range(C):
        for h in range(H):
            cm_cols[c * H + h] = h * C + c
    Wg_cm = Wg[:, cm_cols]
    Wgp = np.concatenate([Wg_cm, Wg @ att_src_e], axis=1).astype(np.float32)
    Wrp = np.concatenate([Wr + 0.0, Wg @ att_dst_e], axis=1).astype(np.float32)

    xts = []
    for c in range(NCORE):
        ids = core_nodes[c]
        xs = np.zeros((R, cfg.d_in), np.float32)
        msk = ids >= 0
        xs[msk] = x[ids[msk]]
        xt = np.zeros((cfg.d_in_pad, R), np.float32)
        xt[:cfg.d_in] = xs.T
        xts.append(xt.astype(NP_BF16))

    bg_cm = bg.reshape(H, C).T.flatten()
    bg_b = np.tile(bg_cm, (P, 1)).astype(NP_BF16)
    W3 = cfg.d_head + cfg.H
    brpad_t = np.zeros((P, W3), np.float32)
    brpad_t[:, :cfg.d_head] = np.tile(br.astype(np.float32), (P, 1))

    meta = dict(core_nodes=core_nodes, pos=pos,
                Ks=[list(map(int, K1)), list(map(int, K2)),
                    list(map(int, K3))],
                idx=idx_w, bg_b=bg_b)
    p1_shared = dict(
        W1p=W1pad.astype(NP_BF16), W2=W2.astype(NP_BF16),
        Wgp=Wgp.astype(NP_BF16), Wrp=Wrp.astype(NP_BF16),
        onep=onep.astype(NP_BF16),
        ones1=ones1.astype(NP_BF16), cvec=cvec, b2v=b2t, brpad=brpad_t)
    p1_maps = [dict(xT=xts[c], **p1_shared) for c in range(NCORE)]
    return p1_maps, meta


def make_sentinel_row(cfg: Cfg) -> np.ndarray:
    row = np.zeros(cfg.row_w, NP_BF16)
    row[cfg.d_head:cfg.d_head + cfg.H] = NP_BF16(-200.0)
    return row


def build_p2_maps(cfg: Cfg, meta, gtabs, ress, adsts):
    gtab_full = np.concatenate(gtabs, axis=0)  # [TR, 384] bf16
    sent = make_sentinel_row(cfg)
    gtab_full[0] = sent
    gtab_full[cfg.sentB_pos] = sent
    p2_maps = []
    for c in range(cfg.n_cores):
        ad = adsts[c]  # [R, H] f32, position order
        adt = ad.reshape(cfg.n_batches, P, cfg.H).transpose(1, 0, 2)
        p2_maps.append(dict(
            gtab=gtab_full, idx=meta["idx"][c],
            adt=adt.astype(NP_BF16).copy(),
            resi=ress[c], bgb=meta["bg_b"],
        ))
    return p2_maps


def kernel(**inputs) -> np.ndarray:
    cfg = CFG
    N = cfg.n_nodes
    NCORE = cfg.n_cores
    DH = cfg.d_head

    p1_maps, meta = prep(cfg, **inputs)

    nc1 = build_phase1(cfg)
    r1 = run_bass_kernel_spmd(nc1, p1_maps, core_ids=list(range(NCORE)))
    gtabs = [r1.results[c]["gtab"] for c in range(NCORE)]
    ress = [r1.results[c]["res"] for c in range(NCORE)]
    adsts = [r1.results[c]["adst"] for c in range(NCORE)]

    nc2 = build_phase2(cfg, meta["Ks"])
    p2_maps = build_p2_maps(cfg, meta, gtabs, ress, adsts)
    r2 = run_bass_kernel_spmd(nc2, p2_maps, core_ids=list(range(NCORE)))

    out = np.zeros((N, DH), np.float32)
    for c in range(NCORE):
        ids = meta["core_nodes"][c]
        msk = ids >= 0
        out[ids[msk]] = r2.results[c]["outp"][msk]
    return out


# revision 19
# speedup vs baseline: 4.4194x; 1.0408x over previous
"""Trainium2 Bass kernel for nn_BaselineGAT (LayerNorm + MLP + GATConv).

Strategy (8 NeuronCores, SPMD, host-mediated phase boundary):
  Phase 1 (per core, nodes sharded 6272/core, degree-sorted order):
    LayerNorm folded into the first matmul (stats via ones-matmul + Square),
    MLP 1488->1024->512 with bf16 matmuls (fp32 PSUM accumulate); stats
    via DVE kt-tree sums + one ones-matmul each, centering/scale on DVE,
    software-pipelined so the PE never waits; then row-major heads.
    Writes per node: a packed g-table row
    [g (256, c-major) | a_src (8)] in bf16 (768B rows), res (256) f32,
    a_dst (8) f32.
  Host: concat g-table shards -> full table [50176, 384] bf16; patch two
    sentinel rows (g=0, a_src=-200) at dummy positions 0 and 3R; padding
    gather slots point at a sentinel, so no masking is needed
    (exp(lrelu(-200+a_dst)) ~ 4e-18 and g=0).
  Phase 2 (per core, edges sharded by dst, fused): nodes are sorted by
    total in-degree and dealt round-robin so each batch of 128 dst rows x
    8 cores shares tight slot capacities. The gather table's row order is
    decoupled from position order (rows are only read via indices): rows
    are permuted so low-out-degree nodes sit in single-window regions and
    high-out-degree nodes where all three windows overlap. Edges gather
    src rows from THREE OVERLAPPING 32768-row table windows based at
    0/8704/17408 (int16 gather indices address <=32768 rows); each batch's
    per-window capacities (K1,K2,K3) come from a small exact LP under
    Hall-feasibility constraints, and a provably-feasible greedy routes
    each dst's edges. Per 128-dst batch: gather K1+K2+K3 slots
    (8-slot/1024-row gather calls; the runtime SWDGE ring is fixed at
    1024 descriptors -- larger calls crash on HW), one fused compute
    pass: e=lrelu(a_src+a_dst), ex=exp into the row, msg=g*ex in place
    (c-major keeps the DVE in 2x 16-bit mode), pairwise-tree reduce
    [g|.|ex] -> slot 0, then normalize (bf16), +bg, elu, transpose to
    h-major, +res -> final output rows.
"""

import sys

sys.path.insert(0, "/opt/trn_rl_repo")

from dataclasses import dataclass

import numpy as np
import ml_dtypes

import concourse.bass as bass  # noqa: F401
import concourse.mybir as mybir
import concourse.tile as tile
from concourse import bacc
from concourse.bass_utils import run_bass_kernel_spmd
from concourse.library_config import mlp as mlp_lib

P = 128
F32 = mybir.dt.float32
BF16 = mybir.dt.bfloat16
I16 = mybir.dt.int16
AL = mybir.AluOpType
AF = mybir.ActivationFunctionType
NP_BF16 = ml_dtypes.bfloat16


@dataclass
class Cfg:
    n_nodes: int = 50000
    n_edges: int = 800000
    d_in: int = 1488
    d_hid: int = 1024
    d_out: int = 512
    C: int = 32
    H: int = 8
    n_cores: int = 8
    node_chunk: int = 512   # phase-1 nodes per chunk
    window: int = 32768     # rows addressable by one int16 gather window
    w2base: int = 8704      # base of the middle gather window
    gather_chunk: int = 8   # phase-2 gather slots per dma_gather call
    ring_bytes: int = 16384  # SWDGE descriptor ring (1024 descs)

    @property
    def d_head(self):  # H*C
        return self.C * self.H

    @property
    def d_in_pad(self):
        return ((self.d_in + P - 1) // P) * P

    @property
    def rows_per_core(self):
        nb = (self.n_nodes + P - 1) // P
        nb = ((nb + self.n_cores - 1) // self.n_cores) * self.n_cores
        return nb // self.n_cores * P

    @property
    def n_batches(self):
        return self.rows_per_core // P

    @property
    def table_rows(self):
        return self.rows_per_core * self.n_cores

    @property
    def baseB(self):
        return self.table_rows - self.window

    @property
    def win_bases(self):
        return (0, self.w2base, self.baseB)

    @property
    def sentB_pos(self):
        # table row used as the w2/w3-window sentinel: core 3, row 0
        return 3 * self.rows_per_core

    @property
    def row_w(self):
        # packed table row in bf16: [g 256 | a_src 8 | ex-slot 8 | pad],
        # 256B-multiple for dma_gather: 384 elems = 768B
        return 384

    @property
    def tree_w(self):
        # reduced width: [g 256 | junk 8 | ex 8]
        return self.d_head + 2 * self.H


CFG = Cfg()

_NC_CACHE = {}


# ----------------------------------------------------------------------------
# Phase 1: LayerNorm + MLP + heads (bf16)
# ----------------------------------------------------------------------------

def build_phase1(cfg: Cfg):
    key = ("p1", cfg.n_nodes, cfg.node_chunk)
    if key in _NC_CACHE:
        return _NC_CACHE[key]
    nc = bacc.Bacc("TRN2", target_bir_lowering=False)
    R = cfg.rows_per_core
    KT1 = cfg.d_in_pad // P          # k-tiles layer 1 (12)
    KT2 = cfg.d_hid // P             # k-tiles layer 2 (8)
    KT3 = cfg.d_out // P             # k-tiles layer 3 (4)
    MT1 = cfg.d_hid // P             # m-tiles layer 1 (8)
    MT2 = cfg.d_out // P             # m-tiles layer 2 (4)
    NCH = cfg.node_chunk
    chunk_sizes = [NCH] * (R // NCH)
    if R % NCH:
        assert R % NCH % P == 0
        chunk_sizes.append(R % NCH)
    # split the first chunk small: the pipeline-fill cost (serial stats ->
    # centering chain before the first matmul) scales with chunk size
    if chunk_sizes[0] > P:
        chunk_sizes = [P, chunk_sizes[0] - P] + chunk_sizes[1:]
    W3 = cfg.d_head + cfg.H          # 264
    DH = cfg.d_head

    xT = nc.dram_tensor("xT", [cfg.d_in_pad, R], BF16, kind="ExternalInput")
    W1p = nc.dram_tensor("W1p", [cfg.d_in_pad, cfg.d_hid], BF16, kind="ExternalInput")
    W2 = nc.dram_tensor("W2", [cfg.d_hid, cfg.d_out], BF16, kind="ExternalInput")
    Wgp = nc.dram_tensor("Wgp", [cfg.d_out, W3], BF16, kind="ExternalInput")
    Wrp = nc.dram_tensor("Wrp", [cfg.d_out, W3], BF16, kind="ExternalInput")
    onep = nc.dram_tensor("onep", [8, P], BF16, kind="ExternalInput")
    ones1 = nc.dram_tensor("ones1", [P, 1], BF16, kind="ExternalInput")
    cvec = nc.dram_tensor("cvec", [P, MT1], F32, kind="ExternalInput")
    b2v = nc.dram_tensor("b2v", [P, MT2], F32, kind="ExternalInput")
    brpad = nc.dram_tensor("brpad", [P, W3], F32, kind="ExternalInput")

    gtab = nc.dram_tensor("gtab", [R, cfg.row_w], BF16, kind="ExternalOutput")
    res = nc.dram_tensor("res", [R, DH], F32, kind="ExternalOutput")
    adst = nc.dram_tensor("adst", [R, cfg.H], F32, kind="ExternalOutput")

    inv_din = 1.0 / cfg.d_in

    with tile.TileContext(nc) as tc:
        with (
            tc.tile_pool(name="wpool", bufs=1) as wp,
            tc.tile_pool(name="xpool", bufs=2) as xp,
            tc.tile_pool(name="sqpool", bufs=2) as sqp,
            tc.tile_pool(name="hpool", bufs=2) as hp,
            tc.tile_pool(name="epool", bufs=3) as ep,
            tc.tile_pool(name="stat", bufs=1) as stp,
            tc.tile_pool(name="ps_y", bufs=2, space="PSUM") as ps_y,
            tc.tile_pool(name="ps_s", bufs=1, space="PSUM") as ps_s,
            tc.tile_pool(name="ps_o", bufs=1, space="PSUM") as ps_o,
        ):
            w1_sb = wp.tile([P, KT1, cfg.d_hid], BF16)
            nc.sync.dma_start(w1_sb[:], W1p.rearrange("(kt p) m -> p kt m", p=P))
            w2_sb = wp.tile([P, KT2, cfg.d_out], BF16)
            nc.sync.dma_start(w2_sb[:], W2.rearrange("(kt p) m -> p kt m", p=P))
            wg_sb = wp.tile([P, KT3, W3], BF16)
            nc.sync.dma_start(wg_sb[:], Wgp.rearrange("(kt p) m -> p kt m", p=P))
            wr_sb = wp.tile([P, KT3, W3], BF16)
            nc.sync.dma_start(wr_sb[:], Wrp.rearrange("(kt p) m -> p kt m", p=P))
            onep_sb = wp.tile([8, P], BF16)
            nc.sync.dma_start(onep_sb[:], onep[:])
            ones1_sb = wp.tile([P, 1], BF16)
            nc.sync.dma_start(ones1_sb[:], ones1[:])
            cvec_sb = wp.tile([P, MT1], F32)
            nc.sync.dma_start(cvec_sb[:], cvec[:])
            b2_sb = wp.tile([P, MT2], F32)
            nc.sync.dma_start(b2_sb[:], b2v[:])
            brp_sb = wp.tile([P, W3], F32)
            nc.sync.dma_start(brp_sb[:], brpad[:])

            def stats_part(ns, NCH):
                # ---- load xT chunk [P, KT1, NCH] (bf16)
                xt = xp.tile([P, KT1, NCH], BF16, tag="xt", name=f"xt{ns}")
                nc.sync.dma_start(
                    xt[:], xT.rearrange("(kt p) n -> p kt n", p=P)[:, :, ns:ns + NCH]
                )
                # ---- stats: per-partition kt-tree sums on DVE, then one
                # ones-matmul each for the 128-partition reduction
                xsum = sqp.tile([P, NCH], BF16, tag="xsum")
                xsq = sqp.tile([P, KT1, NCH], BF16, tag="xsq")
                with nc.allow_low_precision(reason="bf16 kt-tree stats; <1e-3"):
                    nc.vector.tensor_tensor(xsum[:], xt[:, 0], xt[:, 1], op=AL.add)
                    for kt in range(2, KT1):
                        nc.vector.tensor_tensor(xsum[:], xsum[:], xt[:, kt],
                                                op=AL.add)
                for kt in range(KT1):
                    nc.scalar.activation(xsq[:, kt], xt[:, kt], AF.Square)
                with nc.allow_low_precision(reason="bf16 kt-tree stats; <1e-3"):
                    k = KT1
                    while k > 1:
                        hh = (k + 1) // 2
                        lo = k - hh
                        nc.vector.tensor_tensor(xsq[:, :lo], xsq[:, :lo],
                                                xsq[:, hh:k], op=AL.add)
                        k = hh
                s1_ps = ps_s.tile([1, NCH], F32, tag="s1")
                s2_ps = ps_s.tile([1, NCH], F32, tag="s2")
                nc.tensor.matmul(s1_ps[:], ones1_sb[:], xsum[:], start=True,
                                 stop=True)
                nc.tensor.matmul(s2_ps[:], ones1_sb[:], xsq[:, 0], start=True,
                                 stop=True)
                # ---- finalize stats: mu, rstd
                mu_bf = stp.tile([8, NCH], BF16, tag="mu")
                nc.vector.memset(mu_bf[:], 0.0)
                nc.vector.tensor_scalar_mul(mu_bf[0:1, :], s1_ps[:], inv_din)
                mu_f = stp.tile([1, NCH], F32, tag="muf")
                nc.vector.tensor_scalar_mul(mu_f[:], s1_ps[:], inv_din)
                musq = stp.tile([1, NCH], F32, tag="musq")
                nc.vector.tensor_tensor(musq[:], mu_f[:], mu_f[:], op=AL.mult)
                var = stp.tile([1, NCH], F32, tag="var")
                nc.vector.tensor_scalar(var[:], s2_ps[:], inv_din, None, op0=AL.mult)
                nc.vector.tensor_tensor(var[:], var[:], musq[:], op=AL.subtract)
                nc.vector.tensor_scalar_add(var[:], var[:], 1e-5)
                sd = stp.tile([1, NCH], F32, tag="sd")
                nc.scalar.activation(sd[:], var[:], AF.Sqrt)
                rstd = stp.tile([8, NCH], BF16, tag="rstd")
                nc.vector.memset(rstd[:], 0.0)
                with nc.allow_low_precision(
                        reason="rstd broadcast via bf16 matmul; 0.4% scale ok"):
                    nc.vector.reciprocal(rstd[0:1, :], sd[:])
                # broadcast mu, rstd to [P, NCH] via K=8 matmuls; center+scale
                # x in place on DVE (removes the per-mt mu matmul + y*rstd)
                rb_ps = ps_s.tile([P, NCH], F32, tag="rb")
                nc.tensor.matmul(rb_ps[:], onep_sb[:], rstd[:], start=True, stop=True)
                rstd_b = stp.tile([P, NCH], BF16, tag="rstdb")
                nc.vector.tensor_copy(rstd_b[:], rb_ps[:])
                mb_ps = ps_s.tile([P, NCH], F32, tag="mb")
                nc.tensor.matmul(mb_ps[:], onep_sb[:], mu_bf[:], start=True, stop=True)
                mu_b = stp.tile([P, NCH], BF16, tag="mub")
                nc.vector.tensor_copy(mu_b[:], mb_ps[:])
                with nc.allow_low_precision(reason="bf16 x centering; ~0.2%"):
                    nc.vector.tensor_tensor(
                        xt[:], xt[:],
                        mu_b[:].unsqueeze(1).to_broadcast([P, KT1, NCH]),
                        op=AL.subtract)
                    nc.vector.tensor_tensor(
                        xt[:], xt[:],
                        rstd_b[:].unsqueeze(1).to_broadcast([P, KT1, NCH]),
                        op=AL.mult)
                return xt

            def mlp_part(xt, ns, NCH):
                # ---- layer 1: h = relu(W1p^T xn + c)
                h_sb = hp.tile([P, MT1, NCH], BF16, tag="h")
                for mt in range(MT1):
                    y_ps = ps_y.tile([P, NCH], F32, tag="y")
                    for kt in range(KT1):
                        nc.tensor.matmul(y_ps[:], w1_sb[:, kt, mt * P:(mt + 1) * P],
                                         xt[:, kt], start=(kt == 0), stop=(kt == KT1 - 1))
                    nc.scalar.activation(h_sb[:, mt], y_ps[:], AF.Relu,
                                         bias=cvec_sb[:, mt:mt + 1])

                # ---- layer 2: h2 = W2^T h + b2
                h2_sb = hp.tile([P, MT2, NCH], BF16, tag="h2")
                for mt in range(MT2):
                    y2_ps = ps_y.tile([P, NCH], F32, tag="y")
                    for kt in range(KT2):
                        nc.tensor.matmul(y2_ps[:], w2_sb[:, kt, mt * P:(mt + 1) * P],
                                         h_sb[:, kt], start=(kt == 0), stop=(kt == KT2 - 1))
                    nc.scalar.activation(h2_sb[:, mt], y2_ps[:], AF.Identity,
                                         bias=b2_sb[:, mt:mt + 1])

                # ---- layer 3 (row-major): per 128-node subtile
                for nt in range(NCH // P):
                    g_ps = ps_o.tile([P, W3], F32, tag="gps")
                    r_ps = ps_o.tile([P, W3], F32, tag="rps")
                    for kt in range(KT3):
                        nc.tensor.matmul(g_ps[:], h2_sb[:, kt, nt * P:(nt + 1) * P],
                                         wg_sb[:, kt], start=(kt == 0), stop=(kt == KT3 - 1))
                    for kt in range(KT3):
                        nc.tensor.matmul(r_ps[:], h2_sb[:, kt, nt * P:(nt + 1) * P],
                                         wr_sb[:, kt], start=(kt == 0), stop=(kt == KT3 - 1))
                    gt = ep.tile([P, W3], BF16, tag="gt")
                    nc.vector.tensor_copy(gt[:], g_ps[:])
                    rt = ep.tile([P, W3], F32, tag="rt")
                    nc.vector.tensor_tensor(rt[:], r_ps[:], brp_sb[:], op=AL.add)
                    r0 = ns + nt * P
                    nc.sync.dma_start(gtab[r0:r0 + P, :W3], gt[:])
                    nc.sync.dma_start(res[r0:r0 + P, :], rt[:, :DH])
                    nc.sync.dma_start(adst[r0:r0 + P, :], rt[:, DH:W3])

            # software-pipelined emission: stats(k+1) lands before mlp(k) so
            # the PE never waits on the centering chain
            ns = 0
            pend = None  # (xt, ns, NCH)
            for NCH in chunk_sizes:
                xt = stats_part(ns, NCH)
                if pend is not None:
                    mlp_part(*pend)
                pend = (xt, ns, NCH)
                ns += NCH
            mlp_part(*pend)
    nc.compile()
    _NC_CACHE[key] = nc
    return nc


# ----------------------------------------------------------------------------
# Phase 2: fused edge pass + epilogue
# ----------------------------------------------------------------------------

def build_phase2(cfg: Cfg, Ks: list):
    """Ks[w][b]: per-batch slot capacities for the three overlapping gather
    windows (bases 0/8704/17408, width 32768). Joint layout per batch:
    [w1 | w2 | w3] slots, one fused compute pass over all of them."""
    key = ("p2", cfg.n_nodes, tuple(map(tuple, Ks)))
    if key in _NC_CACHE:
        return _NC_CACHE[key]
    nc = bacc.Bacc("TRN2", target_bir_lowering=False,
                   dynamic_dma_scratch_size=cfg.ring_bytes)
    R = cfg.rows_per_core
    NB = cfg.n_batches
    RW = cfg.row_w
    TW = cfg.tree_w        # 272
    DH = cfg.d_head        # 256
    H = cfg.H
    C = cfg.C
    GC = cfg.gather_chunk  # 16
    K1, K2, K3 = Ks
    assert len(K1) == NB and len(K2) == NB and len(K3) == NB
    SCtot = [a + b + c for a, b, c in zip(K1, K2, K3)]
    SCmax = max(SCtot)
    cols = 8 * sum(SCtot)

    gtab = nc.dram_tensor("gtab", [cfg.table_rows, RW], BF16, kind="ExternalInput")
    idx = nc.dram_tensor("idx", [P, cols], I16, kind="ExternalInput")
    adt = nc.dram_tensor("adt", [P, NB, H], BF16, kind="ExternalInput")
    resi = nc.dram_tensor("resi", [R, DH], F32, kind="ExternalInput")
    bgb = nc.dram_tensor("bgb", [P, DH], BF16, kind="ExternalInput")
    outp = nc.dram_tensor("outp", [R, DH], F32, kind="ExternalOutput")

    with tile.TileContext(nc) as tc:
        with (
            tc.tile_pool(name="const", bufs=1) as cp,
            tc.tile_pool(name="gath", bufs=3) as gp,
            tc.tile_pool(name="wk", bufs=3) as wk,
            tc.tile_pool(name="resp", bufs=2) as rp,
            tc.tile_pool(name="outp_", bufs=2) as op_,
        ):
            nc.gpsimd.load_library(mlp_lib)
            idx_sb = cp.tile([P, cols], I16)
            nc.sync.dma_start(idx_sb[:], idx[:])
            adt_sb = cp.tile([P, NB, H], BF16)
            nc.sync.dma_start(adt_sb[:], adt[:])
            bg_sb = cp.tile([P, DH], BF16)
            nc.sync.dma_start(bg_sb[:], bgb[:])

            tabs = [gtab[w0:w0 + cfg.window, :] for w0 in cfg.win_bases]

            off = 0  # global slot offset into idx
            for b in range(NB):
                SCb = SCtot[b]
                gt_full = gp.tile([P, SCmax, RW], BF16, tag="gt", name=f"g{b}")
                gt = gt_full[:, :SCb, :]
                res_t = rp.tile([P, DH], F32, tag="res", name=f"res{b}")
                nc.sync.dma_start(
                    res_t[:], resi.rearrange("(b p) w -> p b w", p=P)[:, b])
                for tab_ap, s0, Kh in ((tabs[0], 0, K1[b]),
                                       (tabs[1], K1[b], K2[b]),
                                       (tabs[2], K1[b] + K2[b], K3[b])):
                    for k0 in range(0, Kh, GC):
                        kk = min(GC, Kh - k0)
                        ni = P * kk
                        nc.gpsimd.dma_gather(
                            gt[:, s0 + k0:s0 + k0 + kk, :], tab_ap,
                            idx_sb[:, 8 * (off + k0):8 * (off + k0 + kk)],
                            ni, ni, RW,
                        )
                    off += Kh
                # e = lrelu(a_src + a_dst); ex = exp(e) -> row slot
                e_t = wk.tile([P, SCmax, H], BF16, tag="et")
                nc.vector.tensor_tensor(
                    e_t[:, :SCb], gt[:, :, DH:DH + H],
                    adt_sb[:, b, :].unsqueeze(1).to_broadcast([P, SCb, H]),
                    op=AL.add)
                nc.vector.scalar_tensor_tensor(
                    e_t[:, :SCb], e_t[:, :SCb], 0.2, e_t[:, :SCb],
                    op0=AL.mult, op1=AL.max)
                nc.scalar.activation(gt[:, :, DH + H:TW], e_t[:, :SCb], AF.Exp)
                # msg = g * ex (broadcast ex over C; c-major layout keeps 2x)
                nc.vector.tensor_tensor(
                    gt[:, :, :DH].rearrange("p k (c h) -> p k c h", h=H),
                    gt[:, :, :DH].rearrange("p k (c h) -> p k c h", h=H),
                    gt[:, :, DH + H:TW].unsqueeze(2).to_broadcast(
                        [P, SCb, C, H]),
                    op=AL.mult)
                # pairwise-tree reduce over slots (bf16, packed rows) -> slot 0
                k = SCb
                while k > 1:
                    hh = (k + 1) // 2
                    lo = k - hh
                    nc.vector.tensor_tensor(
                        gt[:, :lo, :TW], gt[:, :lo, :TW],
                        gt[:, hh:k, :TW], op=AL.add)
                    k = hh
                # ---- epilogue for batch b (from gt[:, 0, :TW])
                acc = gt_full[:, 0, :]
                rec = wk.tile([P, H], BF16, tag="rec")
                with nc.allow_low_precision(
                        reason="bf16 alpha-denominator; ~0.4% head scale"):
                    nc.vector.reciprocal(rec[:], acc[:, DH + H:TW])
                    o_cm = op_.tile([P, DH], BF16, tag="ocm")
                    nc.vector.tensor_tensor(
                        o_cm[:].rearrange("p (c h) -> p c h", h=H),
                        acc[:, :DH].rearrange("p (c h) -> p c h", h=H),
                        rec[:].unsqueeze(1).to_broadcast([P, C, H]),
                        op=AL.mult)
                    nc.vector.tensor_tensor(o_cm[:], o_cm[:], bg_sb[:], op=AL.add)
                    zm = wk.tile([P, DH], BF16, tag="zm")
                    nc.vector.tensor_scalar_min(zm[:], o_cm[:], 0.0)
                ez = wk.tile([P, DH], F32, tag="ez")
                nc.scalar.activation(ez[:], zm[:], AF.Exp)
                o_p = op_.tile([P, DH], F32, tag="op")
                nc.vector.scalar_tensor_tensor(o_p[:], o_cm[:], 0.0, ez[:],
                                               op0=AL.max, op1=AL.add)
                # transpose c-major -> h-major, -1, +res in one op
                o_hm = op_.tile([P, DH], F32, tag="ohm")
                nc.vector.scalar_tensor_tensor(
                    o_hm[:].rearrange("p (h c) -> p h c", c=C),
                    o_p[:].rearrange("p (c h) -> p c h", h=H).transpose([0, 2, 1]),
                    -1.0,
                    res_t[:].rearrange("p (h c) -> p h c", c=C),
                    op0=AL.add, op1=AL.add)
                nc.sync.dma_start(
                    outp.rearrange("(b p) w -> p b w", p=P)[:, b], o_hm[:])
    nc.compile()
    _NC_CACHE[key] = nc
    return nc


# ----------------------------------------------------------------------------
# Host-side preparation
# ----------------------------------------------------------------------------

def wrap_idx(lst: np.ndarray) -> np.ndarray:
    """list index i -> sbuf [16-wrap x 8 replication]: [p, col] = lst[col*16 + p%16]."""
    n = len(lst)
    assert n % 16 == 0
    lay = lst.reshape(n // 16, 16).T.copy()
    return np.tile(lay, (8, 1)).astype(np.int16)


def prep(cfg: Cfg, x, edge_index, ln_g, ln_b, W1, b1, W2, b2, Wr, br, Wg,
         att_src, att_dst, bg):
    """Everything host-side: sharding, permutations, idx arrays, weight prep."""
    N = cfg.n_nodes
    R = cfg.rows_per_core
    NB = cfg.n_batches
    NCORE = cfg.n_cores
    TR = cfg.table_rows
    H, C = cfg.H, cfg.C
    W = cfg.window
    baseB = cfg.baseB

    x = np.asarray(x, np.float32)
    ln_g = np.asarray(ln_g, np.float32)
    ln_b = np.asarray(ln_b, np.float32)
    W1 = np.asarray(W1, np.float32)
    b1 = np.asarray(b1, np.float32)
    W2 = np.asarray(W2, np.float32)
    b2 = np.asarray(b2, np.float32)
    Wr = np.asarray(Wr, np.float32)
    br = np.asarray(br, np.float32)
    Wg = np.asarray(Wg, np.float32)
    att_src = np.asarray(att_src, np.float32)
    att_dst = np.asarray(att_dst, np.float32)
    bg = np.asarray(bg, np.float32)

    src = np.asarray(edge_index[0], np.int64)
    dst = np.asarray(edge_index[1], np.int64)
    loops = np.arange(N, dtype=np.int64)
    src = np.concatenate([src, loops])
    dst = np.concatenate([dst, loops])

    deg = np.bincount(dst, minlength=N)  # in-degree incl self loop

    # ---- order: total-degree sort (dummies first), deal blocks round-robin
    keys = np.concatenate([deg, np.full(TR - N, -1, np.int64)])
    nodes = np.concatenate([np.arange(N), np.full(TR - N, -1, np.int64)])
    order = np.argsort(keys, kind="stable")
    sorted_nodes = nodes[order]
    # dummies occupy the lowest sorted positions. Sentinel A lives at table
    # position 0 (= sorted position 0, a dummy). Sentinel B needs a dummy at
    # table position 3R (core 3, row 0) = sorted position 384 (block 3,
    # partition 0): swap a dummy there.
    assert sorted_nodes[0] < 0 and sorted_nodes[1] < 0
    sorted_nodes[1], sorted_nodes[384] = sorted_nodes[384], sorted_nodes[1]

    blocks = sorted_nodes.reshape(TR // P, P)
    core_nodes = [[] for _ in range(NCORE)]
    for j in range(blocks.shape[0]):
        core_nodes[j % NCORE].append(blocks[j])
    core_nodes = [np.concatenate(bl) for bl in core_nodes]
    pos = np.full(N, -1, np.int64)
    for c in range(NCORE):
        ids = core_nodes[c]
        msk = ids >= 0
        pos[ids[msk]] = c * R + np.nonzero(msk)[0]
    assert core_nodes[0][0] < 0 and core_nodes[3][0] < 0
    sentB_local = cfg.sentB_pos - baseB

    # ---- gather-table row permutation: the table is only read via explicit
    # indices, so its row order is free. Put low-out-degree nodes in the
    # single-window regions and high-out-degree nodes where all three
    # windows overlap -- this shrinks the forced-edge maxima in the
    # capacity LP below.
    odeg = np.bincount(src, minlength=N)  # out-degree incl self (>=1)
    od_pos = np.zeros(TR, np.int64)
    od_pos[pos[np.arange(N)]] = odeg      # dummies stay 0
    oorder = np.argsort(od_pos, kind="stable")
    w2b = cfg.w2base                  # 8704
    w2e = w2b + W                     # 41472
    w3b = baseB                       # 17408
    rows_sorted = np.concatenate([
        np.arange(0, w2b), np.arange(w2e, TR),        # 1-window regions
        np.arange(w2b, w3b), np.arange(W, w2e),       # 2-window regions
        np.arange(w3b, W),                            # 3-window region
    ])
    trow = np.empty(TR, np.int64)
    trow[oorder] = rows_sorted
    # sentinels: table row 0 (w1 padding) and row W=32768 (in w2&w3) must
    # hold dummy rows. Row 0 is the lowest-out-degree position = a dummy
    # already; swap a dummy onto row 32768.
    assert od_pos[np.nonzero(trow == 0)[0][0]] == 0
    p_at = int(np.nonzero(trow == W)[0][0])
    if od_pos[p_at] != 0:
        pd = int(np.nonzero((od_pos == 0) & (trow != 0))[0][0])
        trow[p_at], trow[pd] = trow[pd], trow[p_at]

    # ---- window membership (3 overlapping windows) and per-batch caps
    spos = pos[src]
    dpos = pos[dst]
    tsrc = trow[spos]
    # edge class by table row: 0:{1} 1:{1,2} 2:{1,2,3} 3:{2,3} 4:{3}
    cls = np.full(len(src), 2, np.int64)
    cls[tsrc < w2b] = 0
    cls[(tsrc >= w2b) & (tsrc < w3b)] = 1
    cls[(tsrc >= W) & (tsrc < w2e)] = 3
    cls[tsrc >= w2e] = 4

    ccnt = np.zeros((5, TR), np.int64)
    np.add.at(ccnt, (cls, dpos), 1)
    degp = ccnt.sum(0)

    def batch_max(a):
        return a.reshape(NCORE, NB, P).transpose(1, 0, 2).reshape(NB, -1).max(1)

    M1 = batch_max(ccnt[0])
    M3 = batch_max(ccnt[4])
    M12 = batch_max(ccnt[0] + ccnt[1])
    M23 = batch_max(ccnt[3] + ccnt[4])
    M13 = batch_max(ccnt[0] + ccnt[4])
    M = batch_max(degp)

    K1 = np.zeros(NB, np.int64)
    K2 = np.zeros(NB, np.int64)
    K3 = np.zeros(NB, np.int64)
    for b in range(NB):
        best = None
        for k1 in range(int(M1[b]), int(M[b]) + 1):
            for k2 in range(0, int(M[b]) + 1):
                if k1 + k2 < int(M12[b]):
                    continue
                k3 = max(int(M3[b]), int(M23[b]) - k2, int(M13[b]) - k1,
                         int(M[b]) - k1 - k2, 0)
                if best is None or k1 + k2 + k3 < best[0]:
                    best = (k1 + k2 + k3, k1, k2, k3)
        if best is None or best[0] == 0:
            best = (1, 1, 0, 0)
        K1[b], K2[b], K3[b] = best[1], best[2], best[3]

    # ---- per-dst greedy window fill (feasible by the Hall constraints)
    b_of_pos = (np.arange(TR) % R) // P
    K1p, K2p, K3p = K1[b_of_pos], K2[b_of_pos], K3[b_of_pos]
    n1, c12, c123, c23, n3 = ccnt
    x12_1 = np.minimum(c12, K1p - n1)
    x12_2 = c12 - x12_1
    x23_3 = np.minimum(c23, K3p - n3)
    x23_2 = c23 - x23_3
    rem1 = K1p - n1 - x12_1
    rem2 = K2p - x12_2 - x23_2
    rem3 = K3p - n3 - x23_3
    assert (x12_2 >= 0).all() and (x23_2 >= 0).all() and (rem2 >= 0).all()
    y1 = np.minimum(c123, rem1)
    y2 = np.minimum(c123 - y1, rem2)
    y3 = c123 - y1 - y2
    assert (y3 <= rem3).all(), "greedy window fill infeasible"

    # per-edge window: rank within (dst, class), then threshold
    key_c = dpos * 5 + cls
    srt = np.argsort(key_c, kind="stable")
    kk_ = key_c[srt]
    grp_start = np.r_[0, np.nonzero(np.diff(kk_))[0] + 1]
    sizes = np.diff(np.r_[grp_start, len(kk_)])
    within = np.arange(len(kk_)) - np.repeat(grp_start, sizes)
    crank = np.zeros(len(src), np.int64)
    crank[srt] = within

    win = np.zeros(len(src), np.int64)
    win[cls == 0] = 0
    win[cls == 4] = 2
    m = cls == 1
    win[m] = np.where(crank[m] < x12_1[dpos[m]], 0, 1)
    m = cls == 3
    win[m] = np.where(crank[m] < x23_3[dpos[m]], 2, 1)
    m = cls == 2
    win[m] = np.where(
        crank[m] < y1[dpos[m]], 0,
        np.where(crank[m] < (y1 + y2)[dpos[m]], 1, 2))

    li = tsrc - np.array(cfg.win_bases)[win]
    assert li.min() >= 0 and li.max() < W

    # ---- slot assignment within (dst, window) + idx lists
    core = dpos // R
    row = dpos % R
    SCtot = K1 + K2 + K3
    soff = np.concatenate([[0], np.cumsum(SCtot)])
    nslots = int(soff[-1])
    # default padding: w1 slots -> sentinel (row 0), w2/w3 -> row 32768
    sent2 = W - w2b
    sent3 = W - w3b
    base_list = np.zeros(nslots * P, np.int64)
    for b in range(NB):
        s2 = (soff[b] + K1[b]) * P
        s3 = (soff[b] + K1[b] + K2[b]) * P
        sE = soff[b + 1] * P
        base_list[s2:s3] = sent2
        base_list[s3:sE] = sent3
    lists = [base_list.copy() for _ in range(NCORE)]

    key2 = (core * R + row) * 3 + win
    srt = np.argsort(key2, kind="stable")
    kk_ = key2[srt]
    grp_start = np.r_[0, np.nonzero(np.diff(kk_))[0] + 1]
    sizes = np.diff(np.r_[grp_start, len(kk_)])
    within = np.arange(len(kk_)) - np.repeat(grp_start, sizes)
    ks = np.zeros(len(src), np.int64)
    ks[srt] = within
    b_of = row // P
    p_of = row % P
    slot = ks + np.where(win == 0, 0,
                         np.where(win == 1, K1[b_of], (K1 + K2)[b_of]))
    lpos = (soff[b_of] + slot) * P + p_of
    for c in range(NCORE):
        m = core == c
        lists[c][lpos[m]] = li[m]

    idx_w = [wrap_idx(lists[c]) for c in range(NCORE)]

    # ---- phase-1 inputs
    W1p = W1 * ln_g[:, None]
    W1pad = np.zeros((cfg.d_in_pad, cfg.d_hid), np.float32)
    W1pad[:cfg.d_in] = W1p
    cvec_flat = b1 + ln_b @ W1
    cvec = cvec_flat.reshape(cfg.d_hid // P, P).T.astype(np.float32).copy()
    b2t = b2.reshape(cfg.d_out // P, P).T.astype(np.float32).copy()
    onep = np.zeros((8, P), np.float32)
    onep[0] = 1.0
    ones1 = np.ones((P, 1), np.float32)

    att_src_e = np.zeros((cfg.d_head, H), np.float32)
    att_dst_e = np.zeros((cfg.d_head, H), np.float32)
    for h in range(H):
        att_src_e[h * C:(h + 1) * C, h] = att_src[h]
        att_dst_e[h * C:(h + 1) * C, h] = att_dst[h]
    # c-major column permutation for the g table: col c*H+h <- h*C+c
    cm_cols = np.empty(cfg.d_head, np.int64)
    for c in range(C):
        for h in range(H):
            cm_cols[c * H + h] = h * C + c
    Wg_cm = Wg[:, cm_cols]
    Wgp = np.concatenate([Wg_cm, Wg @ att_src_e], axis=1).astype(np.float32)
    Wrp = np.concatenate([Wr + 0.0, Wg @ att_dst_e], axis=1).astype(np.float32)

    xts = []
    for c in range(NCORE):
        ids = core_nodes[c]
        xs = np.zeros((R, cfg.d_in), np.float32)
        msk = ids >= 0
        xs[msk] = x[ids[msk]]
        xt = np.zeros((cfg.d_in_pad, R), np.float32)
        xt[:cfg.d_in] = xs.T
        xts.append(xt.astype(NP_BF16))

    bg_cm = bg.reshape(H, C).T.flatten()
    bg_b = np.tile(bg_cm, (P, 1)).astype(NP_BF16)
    W3 = cfg.d_head + cfg.H
    brpad_t = np.zeros((P, W3), np.float32)
    brpad_t[:, :cfg.d_head] = np.tile(br.astype(np.float32), (P, 1))

    meta = dict(core_nodes=core_nodes, pos=pos, trow=trow,
                Ks=[list(map(int, K1)), list(map(int, K2)),
                    list(map(int, K3))],
                idx=idx_w, bg_b=bg_b)
    p1_shared = dict(
        W1p=W1pad.astype(NP_BF16), W2=W2.astype(NP_BF16),
        Wgp=Wgp.astype(NP_BF16), Wrp=Wrp.astype(NP_BF16),
        onep=onep.astype(NP_BF16),
        ones1=ones1.astype(NP_BF16), cvec=cvec, b2v=b2t, brpad=brpad_t)
    p1_maps = [dict(xT=xts[c], **p1_shared) for c in range(NCORE)]
    return p1_maps, meta


def make_sentinel_row(cfg: Cfg) -> np.ndarray:
    row = np.zeros(cfg.row_w, NP_BF16)
    row[cfg.d_head:cfg.d_head + cfg.H] = NP_BF16(-200.0)
    return row


def build_p2_maps(cfg: Cfg, meta, gtabs, ress, adsts):
    cat = np.concatenate(gtabs, axis=0)        # [TR, 384] bf16, position order
    gtab_full = np.empty_like(cat)
    gtab_full[meta["trow"]] = cat              # permute to table-row order
    sent = make_sentinel_row(cfg)
    gtab_full[0] = sent
    gtab_full[cfg.window] = sent
    p2_maps = []
    for c in range(cfg.n_cores):
        ad = adsts[c]  # [R, H] f32, position order
        adt = ad.reshape(cfg.n_batches, P, cfg.H).transpose(1, 0, 2)
        p2_maps.append(dict(
            gtab=gtab_full, idx=meta["idx"][c],
            adt=adt.astype(NP_BF16).copy(),
            resi=ress[c], bgb=meta["bg_b"],
        ))
    return p2_maps


def kernel(**inputs) -> np.ndarray:
    cfg = CFG
    N = cfg.n_nodes
    NCORE = cfg.n_cores
    DH = cfg.d_head

    p1_maps, meta = prep(cfg, **inputs)

    nc1 = build_phase1(cfg)
    r1 = run_bass_kernel_spmd(nc1, p1_maps, core_ids=list(range(NCORE)))
    gtabs = [r1.results[c]["gtab"] for c in range(NCORE)]
    ress = [r1.results[c]["res"] for c in range(NCORE)]
    adsts = [r1.results[c]["adst"] for c in range(NCORE)]

    nc2 = build_phase2(cfg, meta["Ks"])
    p2_maps = build_p2_maps(cfg, meta, gtabs, ress, adsts)
    r2 = run_bass_kernel_spmd(nc2, p2_maps, core_ids=list(range(NCORE)))

    out = np.zeros((N, DH), np.float32)
    for c in range(NCORE):
        ids = meta["core_nodes"][c]
        msk = ids >= 0
        out[ids[msk]] = r2.results[c]["outp"][msk]
    return out


# revision 22
# speedup vs baseline: 4.4707x; 1.0116x over previous
"""Trainium2 Bass kernel for nn_BaselineGAT (LayerNorm + MLP + GATConv).

Strategy (8 NeuronCores, SPMD, host-mediated phase boundary):
  Phase 1 (per core, nodes sharded 6272/core, degree-sorted order):
    LayerNorm folded into the first matmul (stats via ones-matmul + Square),
    MLP 1488->1024->512 with bf16 matmuls (fp32 PSUM accumulate); stats
    via DVE kt-tree sums + one ones-matmul each, centering/scale on DVE,
    software-pipelined so the PE never waits; then row-major heads.
    Writes per node: a packed g-table row
    [g (256, c-major) | a_src (8)] in bf16 (768B rows), res (256) f32,
    a_dst (8) f32.
  Host: concat g-table shards -> full table [50176, 384] bf16; patch two
    sentinel rows (g=0, a_src=-200) at dummy positions 0 and 3R; padding
    gather slots point at a sentinel, so no masking is needed
    (exp(lrelu(-200+a_dst)) ~ 4e-18 and g=0).
  Phase 2 (per core, edges sharded by dst, fused): nodes are sorted by
    total in-degree and dealt round-robin so each batch of 128 dst rows x
    8 cores shares tight slot capacities. The gather table's row order is
    decoupled from position order (rows are only read via indices): rows
    are permuted so low-out-degree nodes sit in single-window regions and
    high-out-degree nodes where all three windows overlap. Edges gather
    src rows from THREE OVERLAPPING 32768-row table windows based at
    0/8704/17408 (int16 gather indices address <=32768 rows); each batch's
    per-window capacities (K1,K2,K3) come from a small exact LP under
    Hall-feasibility constraints, and a provably-feasible greedy routes
    each dst's edges. Per 128-dst batch: gather K1+K2+K3 slots
    (8-slot/1024-row gather calls; the runtime SWDGE ring is fixed at
    1024 descriptors -- larger calls crash on HW), one fused compute
    pass: e=lrelu(a_src+a_dst), ex=exp into the row, msg=g*ex in place
    (c-major keeps the DVE in 2x 16-bit mode), pairwise-tree reduce
    [g|.|ex] -> slot 0, then normalize (bf16), +bg, elu, transpose to
    h-major, +res -> final output rows.
"""

import sys

sys.path.insert(0, "/opt/trn_rl_repo")

from dataclasses import dataclass

import numpy as np
import ml_dtypes

import concourse.bass as bass  # noqa: F401
import concourse.mybir as mybir
import concourse.tile as tile
from concourse import bacc
from concourse.bass_utils import run_bass_kernel_spmd
from concourse.library_config import mlp as mlp_lib

P = 128
F32 = mybir.dt.float32
BF16 = mybir.dt.bfloat16
I16 = mybir.dt.int16
AL = mybir.AluOpType
AF = mybir.ActivationFunctionType
NP_BF16 = ml_dtypes.bfloat16


@dataclass
class Cfg:
    n_nodes: int = 50000
    n_edges: int = 800000
    d_in: int = 1488
    d_hid: int = 1024
    d_out: int = 512
    C: int = 32
    H: int = 8
    n_cores: int = 8
    node_chunk: int = 512   # phase-1 nodes per chunk
    window: int = 32768     # rows addressable by one int16 gather window
    w2base: int = 8704      # base of the middle gather window
    gather_chunk: int = 8   # phase-2 gather slots per dma_gather call
    ring_bytes: int = 16384  # SWDGE descriptor ring (1024 descs)

    @property
    def d_head(self):  # H*C
        return self.C * self.H

    @property
    def d_in_pad(self):
        return ((self.d_in + P - 1) // P) * P

    @property
    def rows_per_core(self):
        nb = (self.n_nodes + P - 1) // P
        nb = ((nb + self.n_cores - 1) // self.n_cores) * self.n_cores
        return nb // self.n_cores * P

    @property
    def n_batches(self):
        return self.rows_per_core // P

    @property
    def table_rows(self):
        return self.rows_per_core * self.n_cores

    @property
    def baseB(self):
        return self.table_rows - self.window

    @property
    def win_bases(self):
        return (0, self.w2base, self.baseB)

    @property
    def sentB_pos(self):
        # table row used as the w2/w3-window sentinel: core 3, row 0
        return 3 * self.rows_per_core

    @property
    def row_w(self):
        # packed table row in bf16: [g 256 | a_src 8 | ex-slot 8 | pad],
        # 256B-multiple for dma_gather: 384 elems = 768B
        return 384

    @property
    def tree_w(self):
        # reduced width: [g 256 | ex 8] -- exp overwrites the a_src lane
        # after the e-add consumed it, so the tree skips dead columns
        return self.d_head + self.H


CFG = Cfg()

_NC_CACHE = {}


# ----------------------------------------------------------------------------
# Phase 1: LayerNorm + MLP + heads (bf16)
# ----------------------------------------------------------------------------

def build_phase1(cfg: Cfg):
    key = ("p1", cfg.n_nodes, cfg.node_chunk)
    if key in _NC_CACHE:
        return _NC_CACHE[key]
    nc = bacc.Bacc("TRN2", target_bir_lowering=False)
    R = cfg.rows_per_core
    KT1 = cfg.d_in_pad // P          # k-tiles layer 1 (12)
    KT2 = cfg.d_hid // P             # k-tiles layer 2 (8)
    KT3 = cfg.d_out // P             # k-tiles layer 3 (4)
    MT1 = cfg.d_hid // P             # m-tiles layer 1 (8)
    MT2 = cfg.d_out // P             # m-tiles layer 2 (4)
    NCH = cfg.node_chunk
    chunk_sizes = [NCH] * (R // NCH)
    if R % NCH:
        assert R % NCH % P == 0
        chunk_sizes.append(R % NCH)
    # split the first chunk small: the pipeline-fill cost (serial stats ->
    # centering chain before the first matmul) scales with chunk size
    if chunk_sizes[0] > 2 * P:
        h0 = chunk_sizes[0] // 2
        chunk_sizes = [h0, chunk_sizes[0] - h0] + chunk_sizes[1:]
    W3 = cfg.d_head + cfg.H          # 264
    DH = cfg.d_head

    xT = nc.dram_tensor("xT", [cfg.d_in_pad, R], BF16, kind="ExternalInput")
    W1p = nc.dram_tensor("W1p", [cfg.d_in_pad, cfg.d_hid], BF16, kind="ExternalInput")
    W2 = nc.dram_tensor("W2", [cfg.d_hid, cfg.d_out], BF16, kind="ExternalInput")
    Wgp = nc.dram_tensor("Wgp", [cfg.d_out, W3], BF16, kind="ExternalInput")
    Wrp = nc.dram_tensor("Wrp", [cfg.d_out, W3], BF16, kind="ExternalInput")
    onep = nc.dram_tensor("onep", [8, P], BF16, kind="ExternalInput")
    ones1 = nc.dram_tensor("ones1", [P, 1], BF16, kind="ExternalInput")
    cvec = nc.dram_tensor("cvec", [P, MT1], F32, kind="ExternalInput")
    b2v = nc.dram_tensor("b2v", [P, MT2], F32, kind="ExternalInput")
    brpad = nc.dram_tensor("brpad", [P, W3], F32, kind="ExternalInput")

    gtab = nc.dram_tensor("gtab", [R, cfg.row_w], BF16, kind="ExternalOutput")
    res = nc.dram_tensor("res", [R, DH], F32, kind="ExternalOutput")
    adst = nc.dram_tensor("adst", [R, cfg.H], F32, kind="ExternalOutput")

    inv_din = 1.0 / cfg.d_in

    with tile.TileContext(nc) as tc:
        with (
            tc.tile_pool(name="wpool", bufs=1) as wp,
            tc.tile_pool(name="xpool", bufs=2) as xp,
            tc.tile_pool(name="sqpool", bufs=2) as sqp,
            tc.tile_pool(name="hpool", bufs=2) as hp,
            tc.tile_pool(name="epool", bufs=3) as ep,
            tc.tile_pool(name="stat", bufs=1) as stp,
            tc.tile_pool(name="ps_y", bufs=2, space="PSUM") as ps_y,
            tc.tile_pool(name="ps_s", bufs=1, space="PSUM") as ps_s,
            tc.tile_pool(name="ps_o", bufs=1, space="PSUM") as ps_o,
        ):
            # tiny constants first (the stats matmuls need only these)
            onep_sb = wp.tile([8, P], BF16)
            nc.sync.dma_start(onep_sb[:], onep[:])
            ones1_sb = wp.tile([P, 1], BF16)
            nc.sync.dma_start(ones1_sb[:], ones1[:])
            cvec_sb = wp.tile([P, MT1], F32)
            nc.sync.dma_start(cvec_sb[:], cvec[:])
            b2_sb = wp.tile([P, MT2], F32)
            nc.sync.dma_start(b2_sb[:], b2v[:])
            brp_sb = wp.tile([P, W3], F32)
            nc.sync.dma_start(brp_sb[:], brpad[:])

            def stats_part(ns, NCH):
                # ---- load xT chunk [P, KT1, NCH] (bf16)
                xt = xp.tile([P, KT1, NCH], BF16, tag="xt", name=f"xt{ns}")
                nc.sync.dma_start(
                    xt[:], xT.rearrange("(kt p) n -> p kt n", p=P)[:, :, ns:ns + NCH]
                )
                # ---- stats: per-partition kt-tree sums on DVE, then one
                # ones-matmul each for the 128-partition reduction
                xsum = sqp.tile([P, NCH], BF16, tag="xsum")
                xsq = sqp.tile([P, KT1, NCH], BF16, tag="xsq")
                with nc.allow_low_precision(reason="bf16 kt-tree stats; <1e-3"):
                    nc.vector.tensor_tensor(xsum[:], xt[:, 0], xt[:, 1], op=AL.add)
                    for kt in range(2, KT1):
                        nc.vector.tensor_tensor(xsum[:], xsum[:], xt[:, kt],
                                                op=AL.add)
                for kt in range(KT1):
                    nc.scalar.activation(xsq[:, kt], xt[:, kt], AF.Square)
                with nc.allow_low_precision(reason="bf16 kt-tree stats; <1e-3"):
                    k = KT1
                    while k > 1:
                        hh = (k + 1) // 2
                        lo = k - hh
                        nc.vector.tensor_tensor(xsq[:, :lo], xsq[:, :lo],
                                                xsq[:, hh:k], op=AL.add)
                        k = hh
                s1_ps = ps_s.tile([1, NCH], F32, tag="s1")
                s2_ps = ps_s.tile([1, NCH], F32, tag="s2")
                nc.tensor.matmul(s1_ps[:], ones1_sb[:], xsum[:], start=True,
                                 stop=True)
                nc.tensor.matmul(s2_ps[:], ones1_sb[:], xsq[:, 0], start=True,
                                 stop=True)
                # ---- finalize stats: mu, rstd
                mu_bf = stp.tile([8, NCH], BF16, tag="mu")
                nc.vector.memset(mu_bf[:], 0.0)
                nc.vector.tensor_scalar_mul(mu_bf[0:1, :], s1_ps[:], inv_din)
                mu_f = stp.tile([1, NCH], F32, tag="muf")
                nc.vector.tensor_scalar_mul(mu_f[:], s1_ps[:], inv_din)
                musq = stp.tile([1, NCH], F32, tag="musq")
                nc.vector.tensor_tensor(musq[:], mu_f[:], mu_f[:], op=AL.mult)
                var = stp.tile([1, NCH], F32, tag="var")
                nc.vector.tensor_scalar(var[:], s2_ps[:], inv_din, None, op0=AL.mult)
                nc.vector.tensor_tensor(var[:], var[:], musq[:], op=AL.subtract)
                nc.vector.tensor_scalar_add(var[:], var[:], 1e-5)
                sd = stp.tile([1, NCH], F32, tag="sd")
                nc.scalar.activation(sd[:], var[:], AF.Sqrt)
                rstd = stp.tile([8, NCH], BF16, tag="rstd")
                nc.vector.memset(rstd[:], 0.0)
                with nc.allow_low_precision(
                        reason="rstd broadcast via bf16 matmul; 0.4% scale ok"):
                    nc.vector.reciprocal(rstd[0:1, :], sd[:])
                # broadcast mu, rstd to [P, NCH] via K=8 matmuls; center+scale
                # x in place on DVE (removes the per-mt mu matmul + y*rstd)
                rb_ps = ps_s.tile([P, NCH], F32, tag="rb")
                nc.tensor.matmul(rb_ps[:], onep_sb[:], rstd[:], start=True, stop=True)
                rstd_b = stp.tile([P, NCH], BF16, tag="rstdb")
                nc.vector.tensor_copy(rstd_b[:], rb_ps[:])
                mb_ps = ps_s.tile([P, NCH], F32, tag="mb")
                nc.tensor.matmul(mb_ps[:], onep_sb[:], mu_bf[:], start=True, stop=True)
                mu_b = stp.tile([P, NCH], BF16, tag="mub")
                nc.vector.tensor_copy(mu_b[:], mb_ps[:])
                with nc.allow_low_precision(reason="bf16 x centering; ~0.2%"):
                    nc.vector.tensor_tensor(
                        xt[:], xt[:],
                        mu_b[:].unsqueeze(1).to_broadcast([P, KT1, NCH]),
                        op=AL.subtract)
                    nc.vector.tensor_tensor(
                        xt[:], xt[:],
                        rstd_b[:].unsqueeze(1).to_broadcast([P, KT1, NCH]),
                        op=AL.mult)
                return xt

            def mlp_part(xt, ns, NCH):
                # ---- layer 1: h = relu(W1p^T xn + c)
                h_sb = hp.tile([P, MT1, NCH], BF16, tag="h")
                for mt in range(MT1):
                    y_ps = ps_y.tile([P, NCH], F32, tag="y")
                    for kt in range(KT1):
                        nc.tensor.matmul(y_ps[:], w1_sb[:, kt, mt * P:(mt + 1) * P],
                                         xt[:, kt], start=(kt == 0), stop=(kt == KT1 - 1))
                    nc.scalar.activation(h_sb[:, mt], y_ps[:], AF.Relu,
                                         bias=cvec_sb[:, mt:mt + 1])

                # ---- layer 2: h2 = W2^T h + b2
                h2_sb = hp.tile([P, MT2, NCH], BF16, tag="h2")
                for mt in range(MT2):
                    y2_ps = ps_y.tile([P, NCH], F32, tag="y")
                    for kt in range(KT2):
                        nc.tensor.matmul(y2_ps[:], w2_sb[:, kt, mt * P:(mt + 1) * P],
                                         h_sb[:, kt], start=(kt == 0), stop=(kt == KT2 - 1))
                    nc.scalar.activation(h2_sb[:, mt], y2_ps[:], AF.Identity,
                                         bias=b2_sb[:, mt:mt + 1])

                # ---- layer 3 (row-major): per 128-node subtile
                for nt in range(NCH // P):
                    g_ps = ps_o.tile([P, W3], F32, tag="gps")
                    r_ps = ps_o.tile([P, W3], F32, tag="rps")
                    for kt in range(KT3):
                        nc.tensor.matmul(g_ps[:], h2_sb[:, kt, nt * P:(nt + 1) * P],
                                         wg_sb[:, kt], start=(kt == 0), stop=(kt == KT3 - 1))
                    for kt in range(KT3):
                        nc.tensor.matmul(r_ps[:], h2_sb[:, kt, nt * P:(nt + 1) * P],
                                         wr_sb[:, kt], start=(kt == 0), stop=(kt == KT3 - 1))
                    gt = ep.tile([P, W3], BF16, tag="gt")
                    nc.vector.tensor_copy(gt[:], g_ps[:])
                    rt = ep.tile([P, W3], F32, tag="rt")
                    nc.vector.tensor_tensor(rt[:], r_ps[:], brp_sb[:], op=AL.add)
                    r0 = ns + nt * P
                    nc.sync.dma_start(gtab[r0:r0 + P, :W3], gt[:])
                    nc.sync.dma_start(res[r0:r0 + P, :], rt[:, :DH])
                    nc.sync.dma_start(adst[r0:r0 + P, :], rt[:, DH:W3])

            # software-pipelined emission: stats(k+1) lands before mlp(k) so
            # the PE never waits on the centering chain. Chunk 0's load+stats
            # are emitted BEFORE the big weight DMAs (stats need no weights),
            # with w1 first among the weights, so the ramp overlaps the
            # weight transfers.
            xt0 = stats_part(0, chunk_sizes[0])
            w1_sb = wp.tile([P, KT1, cfg.d_hid], BF16)
            nc.sync.dma_start(w1_sb[:], W1p.rearrange("(kt p) m -> p kt m", p=P))
            w2_sb = wp.tile([P, KT2, cfg.d_out], BF16)
            nc.sync.dma_start(w2_sb[:], W2.rearrange("(kt p) m -> p kt m", p=P))
            wg_sb = wp.tile([P, KT3, W3], BF16)
            nc.sync.dma_start(wg_sb[:], Wgp.rearrange("(kt p) m -> p kt m", p=P))
            wr_sb = wp.tile([P, KT3, W3], BF16)
            nc.sync.dma_start(wr_sb[:], Wrp.rearrange("(kt p) m -> p kt m", p=P))
            ns = chunk_sizes[0]
            pend = (xt0, 0, chunk_sizes[0])  # (xt, ns, NCH)
            for NCH in chunk_sizes[1:]:
                xt = stats_part(ns, NCH)
                mlp_part(*pend)
                pend = (xt, ns, NCH)
                ns += NCH
            mlp_part(*pend)
    nc.compile()
    _NC_CACHE[key] = nc
    return nc


# ----------------------------------------------------------------------------
# Phase 2: fused edge pass + epilogue
# ----------------------------------------------------------------------------

def build_phase2(cfg: Cfg, Ks: list):
    """Ks[w][b]: per-batch slot capacities for the three overlapping gather
    windows (bases 0/8704/17408, width 32768). Joint layout per batch:
    [w1 | w2 | w3] slots, one fused compute pass over all of them."""
    key = ("p2", cfg.n_nodes, tuple(map(tuple, Ks)))
    if key in _NC_CACHE:
        return _NC_CACHE[key]
    nc = bacc.Bacc("TRN2", target_bir_lowering=False,
                   dynamic_dma_scratch_size=cfg.ring_bytes)
    R = cfg.rows_per_core
    NB = cfg.n_batches
    RW = cfg.row_w
    TW = cfg.tree_w        # 272
    DH = cfg.d_head        # 256
    H = cfg.H
    C = cfg.C
    GC = cfg.gather_chunk  # 16
    K1, K2, K3 = Ks
    assert len(K1) == NB and len(K2) == NB and len(K3) == NB
    SCtot = [a + b + c for a, b, c in zip(K1, K2, K3)]
    SCmax = max(SCtot)
    cols = 8 * sum(SCtot)

    gtab = nc.dram_tensor("gtab", [cfg.table_rows, RW], BF16, kind="ExternalInput")
    idx = nc.dram_tensor("idx", [P, cols], I16, kind="ExternalInput")
    adt = nc.dram_tensor("adt", [P, NB, H], BF16, kind="ExternalInput")
    resi = nc.dram_tensor("resi", [R, DH], F32, kind="ExternalInput")
    bgb = nc.dram_tensor("bgb", [P, DH], BF16, kind="ExternalInput")
    outp = nc.dram_tensor("outp", [R, DH], F32, kind="ExternalOutput")

    with tile.TileContext(nc) as tc:
        with (
            tc.tile_pool(name="const", bufs=1) as cp,
            tc.tile_pool(name="gath", bufs=3) as gp,
            tc.tile_pool(name="wk", bufs=3) as wk,
            tc.tile_pool(name="resp", bufs=2) as rp,
            tc.tile_pool(name="outp_", bufs=2) as op_,
        ):
            nc.gpsimd.load_library(mlp_lib)
            idx_sb = cp.tile([P, cols], I16)
            nc.sync.dma_start(idx_sb[:], idx[:])
            adt_sb = cp.tile([P, NB, H], BF16)
            nc.sync.dma_start(adt_sb[:], adt[:])
            bg_sb = cp.tile([P, DH], BF16)
            nc.sync.dma_start(bg_sb[:], bgb[:])

            tabs = [gtab[w0:w0 + cfg.window, :] for w0 in cfg.win_bases]

            off = 0  # global slot offset into idx
            for b in range(NB):
                SCb = SCtot[b]
                gt_full = gp.tile([P, SCmax, RW], BF16, tag="gt", name=f"g{b}")
                gt = gt_full[:, :SCb, :]
                res_t = rp.tile([P, DH], F32, tag="res", name=f"res{b}")
                nc.sync.dma_start(
                    res_t[:], resi.rearrange("(b p) w -> p b w", p=P)[:, b])
                for tab_ap, s0, Kh in ((tabs[0], 0, K1[b]),
                                       (tabs[1], K1[b], K2[b]),
                                       (tabs[2], K1[b] + K2[b], K3[b])):
                    for k0 in range(0, Kh, GC):
                        kk = min(GC, Kh - k0)
                        ni = P * kk
                        nc.gpsimd.dma_gather(
                            gt[:, s0 + k0:s0 + k0 + kk, :], tab_ap,
                            idx_sb[:, 8 * (off + k0):8 * (off + k0 + kk)],
                            ni, ni, RW,
                        )
                    off += Kh
                # e = lrelu(a_src + a_dst); ex = exp(e) -> row slot
                e_t = wk.tile([P, SCmax, H], BF16, tag="et")
                nc.vector.tensor_tensor(
                    e_t[:, :SCb], gt[:, :, DH:DH + H],
                    adt_sb[:, b, :].unsqueeze(1).to_broadcast([P, SCb, H]),
                    op=AL.add)
                nc.vector.scalar_tensor_tensor(
                    e_t[:, :SCb], e_t[:, :SCb], 0.2, e_t[:, :SCb],
                    op0=AL.mult, op1=AL.max)
                nc.scalar.activation(gt[:, :, DH:TW], e_t[:, :SCb], AF.Exp)
                # msg = g * ex (broadcast ex over C; c-major layout keeps 2x)
                nc.vector.tensor_tensor(
                    gt[:, :, :DH].rearrange("p k (c h) -> p k c h", h=H),
                    gt[:, :, :DH].rearrange("p k (c h) -> p k c h", h=H),
                    gt[:, :, DH:TW].unsqueeze(2).to_broadcast(
                        [P, SCb, C, H]),
                    op=AL.mult)
                # pairwise-tree reduce over slots (bf16, packed rows) -> slot 0
                k = SCb
                while k > 1:
                    hh = (k + 1) // 2
                    lo = k - hh
                    nc.vector.tensor_tensor(
                        gt[:, :lo, :TW], gt[:, :lo, :TW],
                        gt[:, hh:k, :TW], op=AL.add)
                    k = hh
                # ---- epilogue for batch b (from gt[:, 0, :TW])
                acc = gt_full[:, 0, :]
                rec = wk.tile([P, H], BF16, tag="rec")
                with nc.allow_low_precision(
                        reason="bf16 alpha-denominator; ~0.4% head scale"):
                    nc.vector.reciprocal(rec[:], acc[:, DH:TW])
                    o_cm = op_.tile([P, DH], BF16, tag="ocm")
                    nc.vector.tensor_tensor(
                        o_cm[:].rearrange("p (c h) -> p c h", h=H),
                        acc[:, :DH].rearrange("p (c h) -> p c h", h=H),
                        rec[:].unsqueeze(1).to_broadcast([P, C, H]),
                        op=AL.mult)
                    nc.vector.tensor_tensor(o_cm[:], o_cm[:], bg_sb[:], op=AL.add)
                    zm = wk.tile([P, DH], BF16, tag="zm")
                    nc.vector.tensor_scalar_min(zm[:], o_cm[:], 0.0)
                ez = wk.tile([P, DH], F32, tag="ez")
                nc.scalar.activation(ez[:], zm[:], AF.Exp)
                o_p = op_.tile([P, DH], F32, tag="op")
                nc.vector.scalar_tensor_tensor(o_p[:], o_cm[:], 0.0, ez[:],
                                               op0=AL.max, op1=AL.add)
                # transpose c-major -> h-major, -1, +res in one op
                o_hm = op_.tile([P, DH], F32, tag="ohm")
                nc.vector.scalar_tensor_tensor(
                    o_hm[:].rearrange("p (h c) -> p h c", c=C),
                    o_p[:].rearrange("p (c h) -> p c h", h=H).transpose([0, 2, 1]),
                    -1.0,
                    res_t[:].rearrange("p (h c) -> p h c", c=C),
                    op0=AL.add, op1=AL.add)
                nc.sync.dma_start(
                    outp.rearrange("(b p) w -> p b w", p=P)[:, b], o_hm[:])
    nc.compile()
    _NC_CACHE[key] = nc
    return nc


# ----------------------------------------------------------------------------
# Host-side preparation
# ----------------------------------------------------------------------------

def wrap_idx(lst: np.ndarray) -> np.ndarray:
    """list index i -> sbuf [16-wrap x 8 replication]: [p, col] = lst[col*16 + p%16]."""
    n = len(lst)
    assert n % 16 == 0
    lay = lst.reshape(n // 16, 16).T.copy()
    return np.tile(lay, (8, 1)).astype(np.int16)


def prep(cfg: Cfg, x, edge_index, ln_g, ln_b, W1, b1, W2, b2, Wr, br, Wg,
         att_src, att_dst, bg):
    """Everything host-side: sharding, permutations, idx arrays, weight prep."""
    N = cfg.n_nodes
    R = cfg.rows_per_core
    NB = cfg.n_batches
    NCORE = cfg.n_cores
    TR = cfg.table_rows
    H, C = cfg.H, cfg.C
    W = cfg.window
    baseB = cfg.baseB

    x = np.asarray(x, np.float32)
    ln_g = np.asarray(ln_g, np.float32)
    ln_b = np.asarray(ln_b, np.float32)
    W1 = np.asarray(W1, np.float32)
    b1 = np.asarray(b1, np.float32)
    W2 = np.asarray(W2, np.float32)
    b2 = np.asarray(b2, np.float32)
    Wr = np.asarray(Wr, np.float32)
    br = np.asarray(br, np.float32)
    Wg = np.asarray(Wg, np.float32)
    att_src = np.asarray(att_src, np.float32)
    att_dst = np.asarray(att_dst, np.float32)
    bg = np.asarray(bg, np.float32)

    src = np.asarray(edge_index[0], np.int64)
    dst = np.asarray(edge_index[1], np.int64)
    loops = np.arange(N, dtype=np.int64)
    src = np.concatenate([src, loops])
    dst = np.concatenate([dst, loops])

    deg = np.bincount(dst, minlength=N)  # in-degree incl self loop

    # ---- order: total-degree sort (dummies first), deal blocks round-robin
    keys = np.concatenate([deg, np.full(TR - N, -1, np.int64)])
    nodes = np.concatenate([np.arange(N), np.full(TR - N, -1, np.int64)])
    order = np.argsort(keys, kind="stable")
    sorted_nodes = nodes[order]
    # dummies occupy the lowest sorted positions. Sentinel A lives at table
    # position 0 (= sorted position 0, a dummy). Sentinel B needs a dummy at
    # table position 3R (core 3, row 0) = sorted position 384 (block 3,
    # partition 0): swap a dummy there.
    assert sorted_nodes[0] < 0 and sorted_nodes[1] < 0
    sorted_nodes[1], sorted_nodes[384] = sorted_nodes[384], sorted_nodes[1]

    blocks = sorted_nodes.reshape(TR // P, P)
    core_nodes = [[] for _ in range(NCORE)]
    for j in range(blocks.shape[0]):
        core_nodes[j % NCORE].append(blocks[j])
    core_nodes = [np.concatenate(bl) for bl in core_nodes]
    pos = np.full(N, -1, np.int64)
    for c in range(NCORE):
        ids = core_nodes[c]
        msk = ids >= 0
        pos[ids[msk]] = c * R + np.nonzero(msk)[0]
    assert core_nodes[0][0] < 0 and core_nodes[3][0] < 0
    sentB_local = cfg.sentB_pos - baseB

    # ---- gather-table row permutation: the table is only read via explicit
    # indices, so its row order is free. Put low-out-degree nodes in the
    # single-window regions and high-out-degree nodes where all three
    # windows overlap -- this shrinks the forced-edge maxima in the
    # capacity LP below.
    odeg = np.bincount(src, minlength=N)  # out-degree incl self (>=1)
    od_pos = np.zeros(TR, np.int64)
    od_pos[pos[np.arange(N)]] = odeg      # dummies stay 0
    oorder = np.argsort(od_pos, kind="stable")
    w2b = cfg.w2base                  # 8704
    w2e = w2b + W                     # 41472
    w3b = baseB                       # 17408
    rows_sorted = np.concatenate([
        np.arange(0, w2b), np.arange(w2e, TR),        # 1-window regions
        np.arange(w2b, w3b), np.arange(W, w2e),       # 2-window regions
        np.arange(w3b, W),                            # 3-window region
    ])
    trow = np.empty(TR, np.int64)
    trow[oorder] = rows_sorted
    # sentinels: table row 0 (w1 padding) and row W=32768 (in w2&w3) must
    # hold dummy rows. Row 0 is the lowest-out-degree position = a dummy
    # already; swap a dummy onto row 32768.
    assert od_pos[np.nonzero(trow == 0)[0][0]] == 0
    p_at = int(np.nonzero(trow == W)[0][0])
    if od_pos[p_at] != 0:
        pd = int(np.nonzero((od_pos == 0) & (trow != 0))[0][0])
        trow[p_at], trow[pd] = trow[pd], trow[p_at]

    # ---- window membership (3 overlapping windows) and per-batch caps
    spos = pos[src]
    dpos = pos[dst]
    tsrc = trow[spos]
    # edge class by table row: 0:{1} 1:{1,2} 2:{1,2,3} 3:{2,3} 4:{3}
    cls = np.full(len(src), 2, np.int64)
    cls[tsrc < w2b] = 0
    cls[(tsrc >= w2b) & (tsrc < w3b)] = 1
    cls[(tsrc >= W) & (tsrc < w2e)] = 3
    cls[tsrc >= w2e] = 4

    ccnt = np.zeros((5, TR), np.int64)
    np.add.at(ccnt, (cls, dpos), 1)
    degp = ccnt.sum(0)

    def batch_max(a):
        return a.reshape(NCORE, NB, P).transpose(1, 0, 2).reshape(NB, -1).max(1)

    M1 = batch_max(ccnt[0])
    M3 = batch_max(ccnt[4])
    M12 = batch_max(ccnt[0] + ccnt[1])
    M23 = batch_max(ccnt[3] + ccnt[4])
    M13 = batch_max(ccnt[0] + ccnt[4])
    M = batch_max(degp)

    K1 = np.zeros(NB, np.int64)
    K2 = np.zeros(NB, np.int64)
    K3 = np.zeros(NB, np.int64)
    for b in range(NB):
        best = None
        for k1 in range(int(M1[b]), int(M[b]) + 1):
            for k2 in range(0, int(M[b]) + 1):
                if k1 + k2 < int(M12[b]):
                    continue
                k3 = max(int(M3[b]), int(M23[b]) - k2, int(M13[b]) - k1,
                         int(M[b]) - k1 - k2, 0)
                if best is None or k1 + k2 + k3 < best[0]:
                    best = (k1 + k2 + k3, k1, k2, k3)
        if best is None or best[0] == 0:
            best = (1, 1, 0, 0)
        K1[b], K2[b], K3[b] = best[1], best[2], best[3]

    # ---- per-dst greedy window fill (feasible by the Hall constraints)
    b_of_pos = (np.arange(TR) % R) // P
    K1p, K2p, K3p = K1[b_of_pos], K2[b_of_pos], K3[b_of_pos]
    n1, c12, c123, c23, n3 = ccnt
    x12_1 = np.minimum(c12, K1p - n1)
    x12_2 = c12 - x12_1
    x23_3 = np.minimum(c23, K3p - n3)
    x23_2 = c23 - x23_3
    rem1 = K1p - n1 - x12_1
    rem2 = K2p - x12_2 - x23_2
    rem3 = K3p - n3 - x23_3
    assert (x12_2 >= 0).all() and (x23_2 >= 0).all() and (rem2 >= 0).all()
    y1 = np.minimum(c123, rem1)
    y2 = np.minimum(c123 - y1, rem2)
    y3 = c123 - y1 - y2
    assert (y3 <= rem3).all(), "greedy window fill infeasible"

    # per-edge window: rank within (dst, class), then threshold
    key_c = dpos * 5 + cls
    srt = np.argsort(key_c, kind="stable")
    kk_ = key_c[srt]
    grp_start = np.r_[0, np.nonzero(np.diff(kk_))[0] + 1]
    sizes = np.diff(np.r_[grp_start, len(kk_)])
    within = np.arange(len(kk_)) - np.repeat(grp_start, sizes)
    crank = np.zeros(len(src), np.int64)
    crank[srt] = within

    win = np.zeros(len(src), np.int64)
    win[cls == 0] = 0
    win[cls == 4] = 2
    m = cls == 1
    win[m] = np.where(crank[m] < x12_1[dpos[m]], 0, 1)
    m = cls == 3
    win[m] = np.where(crank[m] < x23_3[dpos[m]], 2, 1)
    m = cls == 2
    win[m] = np.where(
        crank[m] < y1[dpos[m]], 0,
        np.where(crank[m] < (y1 + y2)[dpos[m]], 1, 2))

    li = tsrc - np.array(cfg.win_bases)[win]
    assert li.min() >= 0 and li.max() < W

    # ---- slot assignment within (dst, window) + idx lists
    core = dpos // R
    row = dpos % R
    SCtot = K1 + K2 + K3
    soff = np.concatenate([[0], np.cumsum(SCtot)])
    nslots = int(soff[-1])
    # default padding: w1 slots -> sentinel (row 0), w2/w3 -> row 32768
    sent2 = W - w2b
    sent3 = W - w3b
    base_list = np.zeros(nslots * P, np.int64)
    for b in range(NB):
        s2 = (soff[b] + K1[b]) * P
        s3 = (soff[b] + K1[b] + K2[b]) * P
        sE = soff[b + 1] * P
        base_list[s2:s3] = sent2
        base_list[s3:sE] = sent3
    lists = [base_list.copy() for _ in range(NCORE)]

    key2 = (core * R + row) * 3 + win
    srt = np.argsort(key2, kind="stable")
    kk_ = key2[srt]
    grp_start = np.r_[0, np.nonzero(np.diff(kk_))[0] + 1]
    sizes = np.diff(np.r_[grp_start, len(kk_)])
    within = np.arange(len(kk_)) - np.repeat(grp_start, sizes)
    ks = np.zeros(len(src), np.int64)
    ks[srt] = within
    b_of = row // P
    p_of = row % P
    slot = ks + np.where(win == 0, 0,
                         np.where(win == 1, K1[b_of], (K1 + K2)[b_of]))
    lpos = (soff[b_of] + slot) * P + p_of
    for c in range(NCORE):
        m = core == c
        lists[c][lpos[m]] = li[m]

    idx_w = [wrap_idx(lists[c]) for c in range(NCORE)]

    # ---- phase-1 inputs
    W1p = W1 * ln_g[:, None]
    W1pad = np.zeros((cfg.d_in_pad, cfg.d_hid), np.float32)
    W1pad[:cfg.d_in] = W1p
    cvec_flat = b1 + ln_b @ W1
    cvec = cvec_flat.reshape(cfg.d_hid // P, P).T.astype(np.float32).copy()
    b2t = b2.reshape(cfg.d_out // P, P).T.astype(np.float32).copy()
    onep = np.zeros((8, P), np.float32)
    onep[0] = 1.0
    ones1 = np.ones((P, 1), np.float32)

    att_src_e = np.zeros((cfg.d_head, H), np.float32)
    att_dst_e = np.zeros((cfg.d_head, H), np.float32)
    for h in range(H):
        att_src_e[h * C:(h + 1) * C, h] = att_src[h]
        att_dst_e[h * C:(h + 1) * C, h] = att_dst[h]
    # c-major column permutation for the g table: col c*H+h <- h*C+c
    cm_cols = np.empty(cfg.d_head, np.int64)
    for c in range(C):
        for h in range(H):
            cm_cols[c * H + h] = h * C + c
    Wg_cm = Wg[:, cm_cols]
    Wgp = np.concatenate([Wg_cm, Wg @ att_src_e], axis=1).astype(np.float32)
    Wrp = np.concatenate([Wr + 0.0, Wg @ att_dst_e], axis=1).astype(np.float32)

    xts = []
    for c in range(NCORE):
        ids = core_nodes[c]
        xs = np.zeros((R, cfg.d_in), np.float32)
        msk = ids >= 0
        xs[msk] = x[ids[msk]]
        xt = np.zeros((cfg.d_in_pad, R), np.float32)
        xt[:cfg.d_in] = xs.T
        xts.append(xt.astype(NP_BF16))

    bg_cm = bg.reshape(H, C).T.flatten()
    bg_b = np.tile(bg_cm, (P, 1)).astype(NP_BF16)
    W3 = cfg.d_head + cfg.H
    brpad_t = np.zeros((P, W3), np.float32)
    brpad_t[:, :cfg.d_head] = np.tile(br.astype(np.float32), (P, 1))

    meta = dict(core_nodes=core_nodes, pos=pos, trow=trow,
                Ks=[list(map(int, K1)), list(map(int, K2)),
                    list(map(int, K3))],
                idx=idx_w, bg_b=bg_b)
    p1_shared = dict(
        W1p=W1pad.astype(NP_BF16), W2=W2.astype(NP_BF16),
        Wgp=Wgp.astype(NP_BF16), Wrp=Wrp.astype(NP_BF16),
        onep=onep.astype(NP_BF16),
        ones1=ones1.astype(NP_BF16), cvec=cvec, b2v=b2t, brpad=brpad_t)
    p1_maps = [dict(xT=xts[c], **p1_shared) for c in range(NCORE)]
    return p1_maps, meta


def make_sentinel_row(cfg: Cfg) -> np.ndarray:
    row = np.zeros(cfg.row_w, NP_BF16)
    row[cfg.d_head:cfg.d_head + cfg.H] = NP_BF16(-200.0)
    return row


def build_p2_maps(cfg: Cfg, meta, gtabs, ress, adsts):
    cat = np.concatenate(gtabs, axis=0)        # [TR, 384] bf16, position order
    gtab_full = np.empty_like(cat)
    gtab_full[meta["trow"]] = cat              # permute to table-row order
    sent = make_sentinel_row(cfg)
    gtab_full[0] = sent
    gtab_full[cfg.window] = sent
    p2_maps = []
    for c in range(cfg.n_cores):
        ad = adsts[c]  # [R, H] f32, position order
        adt = ad.reshape(cfg.n_batches, P, cfg.H).transpose(1, 0, 2)
        p2_maps.append(dict(
            gtab=gtab_full, idx=meta["idx"][c],
            adt=adt.astype(NP_BF16).copy(),
            resi=ress[c], bgb=meta["bg_b"],
        ))
    return p2_maps


def kernel(**inputs) -> np.ndarray:
    cfg = CFG
    N = cfg.n_nodes
    NCORE = cfg.n_cores
    DH = cfg.d_head

    p1_maps, meta = prep(cfg, **inputs)

    nc1 = build_phase1(cfg)
    r1 = run_bass_kernel_spmd(nc1, p1_maps, core_ids=list(range(NCORE)))
    gtabs = [r1.results[c]["gtab"] for c in range(NCORE)]
    ress = [r1.results[c]["res"] for c in range(NCORE)]
    adsts = [r1.results[c]["adst"] for c in range(NCORE)]

    nc2 = build_phase2(cfg, meta["Ks"])
    p2_maps = build_p2_maps(cfg, meta, gtabs, ress, adsts)
    r2 = run_bass_kernel_spmd(nc2, p2_maps, core_ids=list(range(NCORE)))

    out = np.zeros((N, DH), np.float32)
    for c in range(NCORE):
        ids = meta["core_nodes"][c]
        msk = ids >= 0
        out[ids[msk]] = r2.results[c]["outp"][msk]
    return out


# revision 25
# speedup vs baseline: 4.5826x; 1.0250x over previous
"""Trainium2 Bass kernel for nn_BaselineGAT (LayerNorm + MLP + GATConv).

Strategy (8 NeuronCores, SPMD, host-mediated phase boundary):
  Phase 1 (per core, nodes sharded 6272/core, degree-sorted order):
    LayerNorm folded into the first matmul (stats via ones-matmul + Square),
    MLP 1488->1024->512 with bf16 matmuls (fp32 PSUM accumulate); stats
    via DVE kt-tree sums + one ones-matmul each, centering/scale on DVE,
    software-pipelined so the PE never waits; then row-major heads.
    Writes per node: a packed g-table row
    [g (256, c-major) | a_src (8)] in bf16 (768B rows), res (256) f32,
    a_dst (8) f32.
  Host: concat g-table shards -> full table [50176, 384] bf16; patch two
    sentinel rows (g=0, a_src=-200) at dummy positions 0 and 3R; padding
    gather slots point at a sentinel, so no masking is needed
    (exp(lrelu(-200+a_dst)) ~ 4e-18 and g=0).
  Phase 2 (per core, edges sharded by dst, fused): nodes are sorted by
    total in-degree and dealt round-robin so each batch of 128 dst rows x
    8 cores shares tight slot capacities. The gather table's row order is
    decoupled from position order (rows are only read via indices): rows
    are permuted so low-out-degree nodes sit in single-window regions and
    high-out-degree nodes where all three windows overlap. Edges gather
    src rows from FOUR OVERLAPPING 32768-row table windows based at
    0/5803/11605/17408 (int16 gather indices address <=32768 rows); each
    batch's per-window capacities come from a small exact LP under
    Hall-feasibility constraints, and a greedy routes each dst's edges
    (final capacities = max(LP, achieved), so always feasible). Per
    128-dst batch: gather sum-of-K slots
    (8-slot/1024-row gather calls; the runtime SWDGE ring is fixed at
    1024 descriptors -- larger calls crash on HW), one fused compute
    pass: e=lrelu(a_src+a_dst), ex=exp into the row, msg=g*ex in place
    (c-major keeps the DVE in 2x 16-bit mode), pairwise-tree reduce
    [g|.|ex] -> slot 0, then normalize (bf16), +bg, elu, transpose to
    h-major, +res -> final output rows.
"""

import sys

sys.path.insert(0, "/opt/trn_rl_repo")

from dataclasses import dataclass

import numpy as np
import ml_dtypes

import concourse.bass as bass  # noqa: F401
import concourse.mybir as mybir
import concourse.tile as tile
from concourse import bacc
from concourse.bass_utils import run_bass_kernel_spmd
from concourse.library_config import mlp as mlp_lib

P = 128
F32 = mybir.dt.float32
BF16 = mybir.dt.bfloat16
I16 = mybir.dt.int16
AL = mybir.AluOpType
AF = mybir.ActivationFunctionType
NP_BF16 = ml_dtypes.bfloat16


@dataclass
class Cfg:
    n_nodes: int = 50000
    n_edges: int = 800000
    d_in: int = 1488
    d_hid: int = 1024
    d_out: int = 512
    C: int = 32
    H: int = 8
    n_cores: int = 8
    node_chunk: int = 512   # phase-1 nodes per chunk
    window: int = 32768     # rows addressable by one int16 gather window
    gather_chunk: int = 8   # phase-2 gather slots per dma_gather call
    ring_bytes: int = 16384  # SWDGE descriptor ring (1024 descs)

    @property
    def d_head(self):  # H*C
        return self.C * self.H

    @property
    def d_in_pad(self):
        return ((self.d_in + P - 1) // P) * P

    @property
    def rows_per_core(self):
        nb = (self.n_nodes + P - 1) // P
        nb = ((nb + self.n_cores - 1) // self.n_cores) * self.n_cores
        return nb // self.n_cores * P

    @property
    def n_batches(self):
        return self.rows_per_core // P

    @property
    def table_rows(self):
        return self.rows_per_core * self.n_cores

    @property
    def baseB(self):
        return self.table_rows - self.window

    @property
    def win_bases(self):
        return (0, 5803, 11605, self.baseB)

    @property
    def sentB_pos(self):
        # table row used as the w2/w3-window sentinel: core 3, row 0
        return 3 * self.rows_per_core

    @property
    def row_w(self):
        # packed table row in bf16: [g 256 | a_src 8 | ex-slot 8 | pad],
        # 256B-multiple for dma_gather: 384 elems = 768B
        return 384

    @property
    def tree_w(self):
        # reduced width: [g 256 | ex 8] -- exp overwrites the a_src lane
        # after the e-add consumed it, so the tree skips dead columns
        return self.d_head + self.H


CFG = Cfg()

_NC_CACHE = {}


# ----------------------------------------------------------------------------
# Phase 1: LayerNorm + MLP + heads (bf16)
# ----------------------------------------------------------------------------

def build_phase1(cfg: Cfg):
    key = ("p1", cfg.n_nodes, cfg.node_chunk)
    if key in _NC_CACHE:
        return _NC_CACHE[key]
    nc = bacc.Bacc("TRN2", target_bir_lowering=False)
    R = cfg.rows_per_core
    KT1 = cfg.d_in_pad // P          # k-tiles layer 1 (12)
    KT2 = cfg.d_hid // P             # k-tiles layer 2 (8)
    KT3 = cfg.d_out // P             # k-tiles layer 3 (4)
    MT1 = cfg.d_hid // P             # m-tiles layer 1 (8)
    MT2 = cfg.d_out // P             # m-tiles layer 2 (4)
    NCH = cfg.node_chunk
    chunk_sizes = [NCH] * (R // NCH)
    if R % NCH:
        assert R % NCH % P == 0
        chunk_sizes.append(R % NCH)
    # split the first chunk small: the pipeline-fill cost (serial stats ->
    # centering chain before the first matmul) scales with chunk size
    if chunk_sizes[0] > 2 * P:
        h0 = chunk_sizes[0] // 2
        chunk_sizes = [h0, chunk_sizes[0] - h0] + chunk_sizes[1:]
    W3 = cfg.d_head + cfg.H          # 264
    DH = cfg.d_head

    xT = nc.dram_tensor("xT", [cfg.d_in_pad, R], BF16, kind="ExternalInput")
    W1p = nc.dram_tensor("W1p", [cfg.d_in_pad, cfg.d_hid], BF16, kind="ExternalInput")
    W2 = nc.dram_tensor("W2", [cfg.d_hid, cfg.d_out], BF16, kind="ExternalInput")
    Wgp = nc.dram_tensor("Wgp", [cfg.d_out, W3], BF16, kind="ExternalInput")
    Wrp = nc.dram_tensor("Wrp", [cfg.d_out, W3], BF16, kind="ExternalInput")
    onep = nc.dram_tensor("onep", [8, P], BF16, kind="ExternalInput")
    ones1 = nc.dram_tensor("ones1", [P, 1], BF16, kind="ExternalInput")
    cvec = nc.dram_tensor("cvec", [P, MT1], F32, kind="ExternalInput")
    b2v = nc.dram_tensor("b2v", [P, MT2], F32, kind="ExternalInput")
    brpad = nc.dram_tensor("brpad", [P, W3], F32, kind="ExternalInput")

    gtab = nc.dram_tensor("gtab", [R, cfg.row_w], BF16, kind="ExternalOutput")
    res = nc.dram_tensor("res", [R, DH], F32, kind="ExternalOutput")
    adst = nc.dram_tensor("adst", [R, cfg.H], F32, kind="ExternalOutput")

    inv_din = 1.0 / cfg.d_in

    with tile.TileContext(nc) as tc:
        with (
            tc.tile_pool(name="wpool", bufs=1) as wp,
            tc.tile_pool(name="xpool", bufs=2) as xp,
            tc.tile_pool(name="sqpool", bufs=2) as sqp,
            tc.tile_pool(name="hpool", bufs=2) as hp,
            tc.tile_pool(name="epool", bufs=3) as ep,
            tc.tile_pool(name="stat", bufs=1) as stp,
            tc.tile_pool(name="ps_y", bufs=2, space="PSUM") as ps_y,
            tc.tile_pool(name="ps_s", bufs=1, space="PSUM") as ps_s,
            tc.tile_pool(name="ps_o", bufs=1, space="PSUM") as ps_o,
        ):
            # tiny constants first (the stats matmuls need only these)
            onep_sb = wp.tile([8, P], BF16)
            nc.sync.dma_start(onep_sb[:], onep[:])
            ones1_sb = wp.tile([P, 1], BF16)
            nc.sync.dma_start(ones1_sb[:], ones1[:])
            cvec_sb = wp.tile([P, MT1], F32)
            nc.sync.dma_start(cvec_sb[:], cvec[:])
            b2_sb = wp.tile([P, MT2], F32)
            nc.sync.dma_start(b2_sb[:], b2v[:])
            brp_sb = wp.tile([P, W3], F32)
            nc.sync.dma_start(brp_sb[:], brpad[:])

            def stats_part(ns, NCH):
                # ---- load xT chunk [P, KT1, NCH] (bf16)
                xt = xp.tile([P, KT1, NCH], BF16, tag="xt", name=f"xt{ns}")
                nc.sync.dma_start(
                    xt[:], xT.rearrange("(kt p) n -> p kt n", p=P)[:, :, ns:ns + NCH]
                )
                # ---- stats: per-partition kt-tree sums on DVE, then one
                # ones-matmul each for the 128-partition reduction
                xsum = sqp.tile([P, NCH], BF16, tag="xsum")
                xsq = sqp.tile([P, KT1, NCH], BF16, tag="xsq")
                with nc.allow_low_precision(reason="bf16 kt-tree stats; <1e-3"):
                    nc.vector.tensor_tensor(xsum[:], xt[:, 0], xt[:, 1], op=AL.add)
                    for kt in range(2, KT1):
                        nc.vector.tensor_tensor(xsum[:], xsum[:], xt[:, kt],
                                                op=AL.add)
                for kt in range(KT1):
                    nc.scalar.activation(xsq[:, kt], xt[:, kt], AF.Square)
                with nc.allow_low_precision(reason="bf16 kt-tree stats; <1e-3"):
                    k = KT1
                    while k > 1:
                        hh = (k + 1) // 2
                        lo = k - hh
                        nc.vector.tensor_tensor(xsq[:, :lo], xsq[:, :lo],
                                                xsq[:, hh:k], op=AL.add)
                        k = hh
                s1_ps = ps_s.tile([1, NCH], F32, tag="s1")
                s2_ps = ps_s.tile([1, NCH], F32, tag="s2")
                nc.tensor.matmul(s1_ps[:], ones1_sb[:], xsum[:], start=True,
                                 stop=True)
                nc.tensor.matmul(s2_ps[:], ones1_sb[:], xsq[:, 0], start=True,
                                 stop=True)
                # ---- finalize stats: mu, rstd
                mu_bf = stp.tile([8, NCH], BF16, tag="mu")
                nc.vector.memset(mu_bf[:], 0.0)
                nc.vector.tensor_scalar_mul(mu_bf[0:1, :], s1_ps[:], inv_din)
                mu_f = stp.tile([1, NCH], F32, tag="muf")
                nc.vector.tensor_scalar_mul(mu_f[:], s1_ps[:], inv_din)
                musq = stp.tile([1, NCH], F32, tag="musq")
                nc.vector.tensor_tensor(musq[:], mu_f[:], mu_f[:], op=AL.mult)
                var = stp.tile([1, NCH], F32, tag="var")
                nc.vector.tensor_scalar(var[:], s2_ps[:], inv_din, None, op0=AL.mult)
                nc.vector.tensor_tensor(var[:], var[:], musq[:], op=AL.subtract)
                nc.vector.tensor_scalar_add(var[:], var[:], 1e-5)
                sd = stp.tile([1, NCH], F32, tag="sd")
                nc.scalar.activation(sd[:], var[:], AF.Sqrt)
                rstd = stp.tile([8, NCH], BF16, tag="rstd")
                nc.vector.memset(rstd[:], 0.0)
                with nc.allow_low_precision(
                        reason="rstd broadcast via bf16 matmul; 0.4% scale ok"):
                    nc.vector.reciprocal(rstd[0:1, :], sd[:])
                # broadcast mu, rstd to [P, NCH] via K=8 matmuls; center+scale
                # x in place on DVE (removes the per-mt mu matmul + y*rstd)
                rb_ps = ps_s.tile([P, NCH], F32, tag="rb")
                nc.tensor.matmul(rb_ps[:], onep_sb[:], rstd[:], start=True, stop=True)
                rstd_b = stp.tile([P, NCH], BF16, tag="rstdb")
                nc.vector.tensor_copy(rstd_b[:], rb_ps[:])
                mb_ps = ps_s.tile([P, NCH], F32, tag="mb")
                nc.tensor.matmul(mb_ps[:], onep_sb[:], mu_bf[:], start=True, stop=True)
                mu_b = stp.tile([P, NCH], BF16, tag="mub")
                nc.vector.tensor_copy(mu_b[:], mb_ps[:])
                with nc.allow_low_precision(reason="bf16 x centering; ~0.2%"):
                    nc.vector.tensor_tensor(
                        xt[:], xt[:],
                        mu_b[:].unsqueeze(1).to_broadcast([P, KT1, NCH]),
                        op=AL.subtract)
                    nc.vector.tensor_tensor(
                        xt[:], xt[:],
                        rstd_b[:].unsqueeze(1).to_broadcast([P, KT1, NCH]),
                        op=AL.mult)
                return xt

            def mlp_part(xt, ns, NCH):
                # ---- layer 1: h = relu(W1p^T xn + c)
                h_sb = hp.tile([P, MT1, NCH], BF16, tag="h")
                for mt in range(MT1):
                    y_ps = ps_y.tile([P, NCH], F32, tag="y")
                    for kt in range(KT1):
                        nc.tensor.matmul(y_ps[:], w1_sb[:, kt, mt * P:(mt + 1) * P],
                                         xt[:, kt], start=(kt == 0), stop=(kt == KT1 - 1))
                    nc.scalar.activation(h_sb[:, mt], y_ps[:], AF.Relu,
                                         bias=cvec_sb[:, mt:mt + 1])

                # ---- layer 2: h2 = W2^T h + b2
                h2_sb = hp.tile([P, MT2, NCH], BF16, tag="h2")
                for mt in range(MT2):
                    y2_ps = ps_y.tile([P, NCH], F32, tag="y")
                    for kt in range(KT2):
                        nc.tensor.matmul(y2_ps[:], w2_sb[:, kt, mt * P:(mt + 1) * P],
                                         h_sb[:, kt], start=(kt == 0), stop=(kt == KT2 - 1))
                    nc.scalar.activation(h2_sb[:, mt], y2_ps[:], AF.Identity,
                                         bias=b2_sb[:, mt:mt + 1])

                # ---- layer 3 (row-major): per 128-node subtile
                for nt in range(NCH // P):
                    g_ps = ps_o.tile([P, W3], F32, tag="gps")
                    r_ps = ps_o.tile([P, W3], F32, tag="rps")
                    for kt in range(KT3):
                        nc.tensor.matmul(g_ps[:], h2_sb[:, kt, nt * P:(nt + 1) * P],
                                         wg_sb[:, kt], start=(kt == 0), stop=(kt == KT3 - 1))
                    for kt in range(KT3):
                        nc.tensor.matmul(r_ps[:], h2_sb[:, kt, nt * P:(nt + 1) * P],
                                         wr_sb[:, kt], start=(kt == 0), stop=(kt == KT3 - 1))
                    gt = ep.tile([P, W3], BF16, tag="gt")
                    nc.vector.tensor_copy(gt[:], g_ps[:])
                    rt = ep.tile([P, W3], F32, tag="rt")
                    nc.vector.tensor_tensor(rt[:], r_ps[:], brp_sb[:], op=AL.add)
                    r0 = ns + nt * P
                    nc.sync.dma_start(gtab[r0:r0 + P, :W3], gt[:])
                    nc.sync.dma_start(res[r0:r0 + P, :], rt[:, :DH])
                    nc.sync.dma_start(adst[r0:r0 + P, :], rt[:, DH:W3])

            # software-pipelined emission: stats(k+1) lands before mlp(k) so
            # the PE never waits on the centering chain. Chunk 0's load+stats
            # are emitted BEFORE the big weight DMAs (stats need no weights),
            # with w1 first among the weights, so the ramp overlaps the
            # weight transfers.
            xt0 = stats_part(0, chunk_sizes[0])
            w1_sb = wp.tile([P, KT1, cfg.d_hid], BF16)
            nc.sync.dma_start(w1_sb[:], W1p.rearrange("(kt p) m -> p kt m", p=P))
            w2_sb = wp.tile([P, KT2, cfg.d_out], BF16)
            nc.sync.dma_start(w2_sb[:], W2.rearrange("(kt p) m -> p kt m", p=P))
            wg_sb = wp.tile([P, KT3, W3], BF16)
            nc.sync.dma_start(wg_sb[:], Wgp.rearrange("(kt p) m -> p kt m", p=P))
            wr_sb = wp.tile([P, KT3, W3], BF16)
            nc.sync.dma_start(wr_sb[:], Wrp.rearrange("(kt p) m -> p kt m", p=P))
            ns = chunk_sizes[0]
            pend = (xt0, 0, chunk_sizes[0])  # (xt, ns, NCH)
            for NCH in chunk_sizes[1:]:
                xt = stats_part(ns, NCH)
                mlp_part(*pend)
                pend = (xt, ns, NCH)
                ns += NCH
            mlp_part(*pend)
    nc.compile()
    _NC_CACHE[key] = nc
    return nc


# ----------------------------------------------------------------------------
# Phase 2: fused edge pass + epilogue
# ----------------------------------------------------------------------------

def build_phase2(cfg: Cfg, Ks: list):
    """Ks[w][b]: per-batch slot capacities for the overlapping gather
    windows (cfg.win_bases, width 32768). Joint layout per batch:
    [w0 | w1 | ...] slots, one fused compute pass over all of them."""
    key = ("p2", cfg.n_nodes, tuple(map(tuple, Ks)))
    if key in _NC_CACHE:
        return _NC_CACHE[key]
    nc = bacc.Bacc("TRN2", target_bir_lowering=False,
                   dynamic_dma_scratch_size=cfg.ring_bytes)
    R = cfg.rows_per_core
    NB = cfg.n_batches
    RW = cfg.row_w
    TW = cfg.tree_w        # 272
    DH = cfg.d_head        # 256
    H = cfg.H
    C = cfg.C
    GC = cfg.gather_chunk
    assert all(len(k) == NB for k in Ks)
    SCtot = [sum(k[b] for k in Ks) for b in range(NB)]
    SCmax = max(SCtot)
    cols = 8 * sum(SCtot)

    gtab = nc.dram_tensor("gtab", [cfg.table_rows, RW], BF16, kind="ExternalInput")
    idx = nc.dram_tensor("idx", [P, cols], I16, kind="ExternalInput")
    adt = nc.dram_tensor("adt", [P, NB, H], BF16, kind="ExternalInput")
    resi = nc.dram_tensor("resi", [R, DH], F32, kind="ExternalInput")
    bgb = nc.dram_tensor("bgb", [P, DH], BF16, kind="ExternalInput")
    outp = nc.dram_tensor("outp", [R, DH], F32, kind="ExternalOutput")

    with tile.TileContext(nc) as tc:
        with (
            tc.tile_pool(name="const", bufs=1) as cp,
            tc.tile_pool(name="gath", bufs=3) as gp,
            tc.tile_pool(name="wk", bufs=3) as wk,
            tc.tile_pool(name="resp", bufs=2) as rp,
            tc.tile_pool(name="outp_", bufs=2) as op_,
        ):
            nc.gpsimd.load_library(mlp_lib)
            idx_sb = cp.tile([P, cols], I16)
            nc.sync.dma_start(idx_sb[:], idx[:])
            adt_sb = cp.tile([P, NB, H], BF16)
            nc.sync.dma_start(adt_sb[:], adt[:])
            bg_sb = cp.tile([P, DH], BF16)
            nc.sync.dma_start(bg_sb[:], bgb[:])

            tabs = [gtab[w0:w0 + cfg.window, :] for w0 in cfg.win_bases]

            off = 0  # global slot offset into idx
            for b in range(NB):
                SCb = SCtot[b]
                gt_full = gp.tile([P, SCmax, RW], BF16, tag="gt", name=f"g{b}")
                gt = gt_full[:, :SCb, :]
                res_t = rp.tile([P, DH], F32, tag="res", name=f"res{b}")
                nc.sync.dma_start(
                    res_t[:], resi.rearrange("(b p) w -> p b w", p=P)[:, b])
                s0s = [sum(k[b] for k in Ks[:w]) for w in range(len(Ks))]
                for tab_ap, s0, Kh in [(tabs[w], s0s[w], Ks[w][b])
                                       for w in range(len(Ks))]:
                    for k0 in range(0, Kh, GC):
                        kk = min(GC, Kh - k0)
                        ni = P * kk
                        nc.gpsimd.dma_gather(
                            gt[:, s0 + k0:s0 + k0 + kk, :], tab_ap,
                            idx_sb[:, 8 * (off + k0):8 * (off + k0 + kk)],
                            ni, ni, RW,
                        )
                    off += Kh
                # e = lrelu(a_src + a_dst); ex = exp(e) -> row slot
                e_t = wk.tile([P, SCmax, H], BF16, tag="et")
                nc.vector.tensor_tensor(
                    e_t[:, :SCb], gt[:, :, DH:DH + H],
                    adt_sb[:, b, :].unsqueeze(1).to_broadcast([P, SCb, H]),
                    op=AL.add)
                nc.vector.scalar_tensor_tensor(
                    e_t[:, :SCb], e_t[:, :SCb], 0.2, e_t[:, :SCb],
                    op0=AL.mult, op1=AL.max)
                nc.scalar.activation(gt[:, :, DH:TW], e_t[:, :SCb], AF.Exp)
                # msg = g * ex (broadcast ex over C; c-major layout keeps 2x)
                nc.vector.tensor_tensor(
                    gt[:, :, :DH].rearrange("p k (c h) -> p k c h", h=H),
                    gt[:, :, :DH].rearrange("p k (c h) -> p k c h", h=H),
                    gt[:, :, DH:TW].unsqueeze(2).to_broadcast(
                        [P, SCb, C, H]),
                    op=AL.mult)
                # pairwise-tree reduce over slots (bf16, packed rows) -> slot 0
                k = SCb
                while k > 1:
                    hh = (k + 1) // 2
                    lo = k - hh
                    nc.vector.tensor_tensor(
                        gt[:, :lo, :TW], gt[:, :lo, :TW],
                        gt[:, hh:k, :TW], op=AL.add)
                    k = hh
                # ---- epilogue for batch b (from gt[:, 0, :TW])
                acc = gt_full[:, 0, :]
                rec = wk.tile([P, H], BF16, tag="rec")
                with nc.allow_low_precision(
                        reason="bf16 alpha-denominator; ~0.4% head scale"):
                    nc.vector.reciprocal(rec[:], acc[:, DH:TW])
                    o_cm = op_.tile([P, DH], BF16, tag="ocm")
                    nc.vector.tensor_tensor(
                        o_cm[:].rearrange("p (c h) -> p c h", h=H),
                        acc[:, :DH].rearrange("p (c h) -> p c h", h=H),
                        rec[:].unsqueeze(1).to_broadcast([P, C, H]),
                        op=AL.mult)
                    nc.vector.tensor_tensor(o_cm[:], o_cm[:], bg_sb[:], op=AL.add)
                    zm = wk.tile([P, DH], BF16, tag="zm")
                    nc.vector.tensor_scalar_min(zm[:], o_cm[:], 0.0)
                ez = wk.tile([P, DH], F32, tag="ez")
                nc.scalar.activation(ez[:], zm[:], AF.Exp)
                o_p = op_.tile([P, DH], F32, tag="op")
                nc.vector.scalar_tensor_tensor(o_p[:], o_cm[:], 0.0, ez[:],
                                               op0=AL.max, op1=AL.add)
                # transpose c-major -> h-major, -1, +res in one op
                o_hm = op_.tile([P, DH], F32, tag="ohm")
                nc.vector.scalar_tensor_tensor(
                    o_hm[:].rearrange("p (h c) -> p h c", c=C),
                    o_p[:].rearrange("p (c h) -> p c h", h=H).transpose([0, 2, 1]),
                    -1.0,
                    res_t[:].rearrange("p (h c) -> p h c", c=C),
                    op0=AL.add, op1=AL.add)
                nc.sync.dma_start(
                    outp.rearrange("(b p) w -> p b w", p=P)[:, b], o_hm[:])
    nc.compile()
    _NC_CACHE[key] = nc
    return nc


# ----------------------------------------------------------------------------
# Host-side preparation
# ----------------------------------------------------------------------------

def wrap_idx(lst: np.ndarray) -> np.ndarray:
    """list index i -> sbuf [16-wrap x 8 replication]: [p, col] = lst[col*16 + p%16]."""
    n = len(lst)
    assert n % 16 == 0
    lay = lst.reshape(n // 16, 16).T.copy()
    return np.tile(lay, (8, 1)).astype(np.int16)


def prep(cfg: Cfg, x, edge_index, ln_g, ln_b, W1, b1, W2, b2, Wr, br, Wg,
         att_src, att_dst, bg):
    """Everything host-side: sharding, permutations, idx arrays, weight prep."""
    N = cfg.n_nodes
    R = cfg.rows_per_core
    NB = cfg.n_batches
    NCORE = cfg.n_cores
    TR = cfg.table_rows
    H, C = cfg.H, cfg.C
    W = cfg.window
    baseB = cfg.baseB

    x = np.asarray(x, np.float32)
    ln_g = np.asarray(ln_g, np.float32)
    ln_b = np.asarray(ln_b, np.float32)
    W1 = np.asarray(W1, np.float32)
    b1 = np.asarray(b1, np.float32)
    W2 = np.asarray(W2, np.float32)
    b2 = np.asarray(b2, np.float32)
    Wr = np.asarray(Wr, np.float32)
    br = np.asarray(br, np.float32)
    Wg = np.asarray(Wg, np.float32)
    att_src = np.asarray(att_src, np.float32)
    att_dst = np.asarray(att_dst, np.float32)
    bg = np.asarray(bg, np.float32)

    src = np.asarray(edge_index[0], np.int64)
    dst = np.asarray(edge_index[1], np.int64)
    loops = np.arange(N, dtype=np.int64)
    src = np.concatenate([src, loops])
    dst = np.concatenate([dst, loops])

    deg = np.bincount(dst, minlength=N)  # in-degree incl self loop

    # ---- order: total-degree sort (dummies first), deal blocks round-robin
    keys = np.concatenate([deg, np.full(TR - N, -1, np.int64)])
    nodes = np.concatenate([np.arange(N), np.full(TR - N, -1, np.int64)])
    order = np.argsort(keys, kind="stable")
    sorted_nodes = nodes[order]
    # dummies occupy the lowest sorted positions. Sentinel A lives at table
    # position 0 (= sorted position 0, a dummy). Sentinel B needs a dummy at
    # table position 3R (core 3, row 0) = sorted position 384 (block 3,
    # partition 0): swap a dummy there.
    assert sorted_nodes[0] < 0 and sorted_nodes[1] < 0
    sorted_nodes[1], sorted_nodes[384] = sorted_nodes[384], sorted_nodes[1]

    blocks = sorted_nodes.reshape(TR // P, P)
    core_nodes = [[] for _ in range(NCORE)]
    for j in range(blocks.shape[0]):
        core_nodes[j % NCORE].append(blocks[j])
    core_nodes = [np.concatenate(bl) for bl in core_nodes]
    pos = np.full(N, -1, np.int64)
    for c in range(NCORE):
        ids = core_nodes[c]
        msk = ids >= 0
        pos[ids[msk]] = c * R + np.nonzero(msk)[0]
    assert core_nodes[0][0] < 0 and core_nodes[3][0] < 0
    sentB_local = cfg.sentB_pos - baseB

    # ---- gather-table row permutation: the table is only read via explicit
    # indices, so its row order is free. Rows covered by fewer gather
    # windows get the lowest-out-degree nodes; heavily-overlapped rows get
    # the high-out-degree nodes. This shrinks forced-edge maxima in the
    # per-batch capacity LP below.
    odeg = np.bincount(src, minlength=N)  # out-degree incl self (>=1)
    od_pos = np.zeros(TR, np.int64)
    od_pos[pos[np.arange(N)]] = odeg      # dummies stay 0
    oorder = np.argsort(od_pos, kind="stable")
    bases = list(cfg.win_bases)
    NW = len(bases)
    cov = np.zeros(TR, np.int64)
    for b0 in bases:
        cov[b0:b0 + W] += 1
    rows_by_cov = np.argsort(cov, kind="stable")
    trow = np.empty(TR, np.int64)
    trow[oorder] = rows_by_cov
    # sentinels: table row 0 (window-0 padding) and row 32768 (inside every
    # other window, outside w0) must hold dummy rows
    assert od_pos[np.nonzero(trow == 0)[0][0]] == 0
    p_at = int(np.nonzero(trow == W)[0][0])
    if od_pos[p_at] != 0:
        pd = int(np.nonzero((od_pos == 0) & (trow != 0))[0][0])
        trow[p_at], trow[pd] = trow[pd], trow[p_at]

    # ---- window membership (interval classes) and per-batch capacities
    spos = pos[src]
    dpos = pos[dst]
    tsrc = trow[spos]
    member = np.stack([(tsrc >= b0) & (tsrc < b0 + W) for b0 in bases])
    assert member.any(0).all()
    first = member.argmax(0)
    last = NW - 1 - member[::-1].argmax(0)

    # per-dst per-class counts; classes are window intervals [i..j]
    iv_ids = {}
    for i in range(NW):
        for j in range(i, NW):
            iv_ids[(i, j)] = len(iv_ids)
    cid = np.array([iv_ids[(a, b)] for a, b in zip(first.tolist(),
                                                   last.tolist())])
    ncls = len(iv_ids)
    ccnt = np.zeros((ncls, TR), np.int64)
    np.add.at(ccnt, (cid, dpos), 1)
    degp = ccnt.sum(0)

    def bmax(a):
        return a.reshape(NCORE, NB, P).transpose(1, 0, 2).reshape(NB, -1).max(1)

    from itertools import combinations
    subs = []
    for r_ in range(1, NW + 1):
        for S in combinations(range(NW), r_):
            subs.append(frozenset(S))
    M_S = {}
    for S in subs:
        tot = np.zeros(TR, np.int64)
        for (i, j), k in iv_ids.items():
            if set(range(i, j + 1)) <= S:
                tot += ccnt[k]
        M_S[S] = bmax(tot)

    # per-batch LP: enumerate K0,K1; K2/K3 in closed form from the subset
    # constraints (K2min from subsets with 2 w/o 3; K3min with 3 w/o 2;
    # K2+K3 from subsets with both)
    Ks = np.zeros((NW, NB), np.int64)
    for b in range(NB):
        Mb = {S: int(M_S[S][b]) for S in subs}
        full = Mb[frozenset(range(NW))]
        m0 = Mb[frozenset({0})]
        best = None
        for k0 in range(m0, full + 1):
            for k1 in range(0, full + 1 - k0):
                ok = True
                for S in subs:
                    if 2 in S or 3 in S:
                        continue
                    if sum((k0, k1)[w] for w in S) < Mb[S]:
                        ok = False
                        break
                if not ok:
                    continue
                k2min, k3min, pair = 0, 0, 0
                for S in subs:
                    rest = sum((k0, k1)[w] for w in S if w < 2)
                    if 2 in S and 3 not in S:
                        k2min = max(k2min, Mb[S] - rest)
                    elif 3 in S and 2 not in S:
                        k3min = max(k3min, Mb[S] - rest)
                    elif 2 in S and 3 in S:
                        pair = max(pair, Mb[S] - rest)
                tot23 = max(k2min + k3min, pair)
                s = k0 + k1 + tot23
                if best is None or s < best[0]:
                    k2 = k2min
                    k3 = tot23 - k2
                    best = (s, k0, k1, k2, k3)
            if best is not None and best[0] == full:
                break
        if best is None:
            best = (1, 1, 0, 0, 0)
        for w in range(NW):
            Ks[w, b] = best[1 + w]

    # ---- per-dst greedy fill guided by the LP capacities; final capacities
    # are the elementwise max of LP and achieved, so feasibility holds by
    # construction
    b_of_pos = (np.arange(TR) % R) // P
    Kp = Ks[:, b_of_pos]                      # [NW, TR]
    used = np.zeros((NW, TR), np.int64)
    win = np.full(len(src), -1, np.int64)
    # narrowest classes first, end-anchored before middle
    order_cls = sorted(iv_ids.items(), key=lambda kv: (kv[0][1] - kv[0][0],
                                                       min(kv[0][0],
                                                           NW - 1 - kv[0][1])))
    # rank edges within (dst, class)
    key_c = dpos * ncls + cid
    srt = np.argsort(key_c, kind="stable")
    kk_ = key_c[srt]
    grp_start = np.r_[0, np.nonzero(np.diff(kk_))[0] + 1]
    sizes = np.diff(np.r_[grp_start, len(kk_)])
    within = np.arange(len(kk_)) - np.repeat(grp_start, sizes)
    crank = np.zeros(len(src), np.int64)
    crank[srt] = within
    for (i, j), k in order_cls:
        m = cid == k
        if not m.any():
            continue
        dp = dpos[m]
        rk = crank[m]
        # fill windows i..j in order: window w takes the next
        # (Kp[w]-used[w]) ranks; order ends-first for end-anchored classes
        worder = list(range(i, j + 1))
        if i == 0 and j < NW - 1:
            worder = list(range(i, j + 1))          # left-anchored: fill left
        elif j == NW - 1 and i > 0:
            worder = list(range(j, i - 1, -1))      # right-anchored: fill right
        assigned = np.full(len(dp), -1, np.int64)
        base = np.zeros(len(dp), np.int64)
        for w in worder[:-1]:
            room = np.maximum(Kp[w, dp] - used[w, dp], 0)
            take = (rk >= base) & (rk < base + room) & (assigned < 0)
            assigned[take] = w
            base = base + room
        lastw = worder[-1]
        assigned[assigned < 0] = lastw
        win[m] = assigned
        np.add.at(used, (assigned, dp), 1)
    assert (win >= 0).all()
    # final capacities: max(LP, achieved)
    for w in range(NW):
        Ks[w] = np.maximum(Ks[w], bmax(used[w]))
    Ks = np.maximum(Ks, 0)
    Ks[0] = np.maximum(Ks[0], 1)

    li = tsrc - np.array(bases)[win]
    assert li.min() >= 0 and li.max() < W

    # ---- slot assignment within (dst, window) + idx lists
    core = dpos // R
    row = dpos % R
    SCtot = Ks.sum(0)
    soff = np.concatenate([[0], np.cumsum(SCtot)])
    nslots = int(soff[-1])
    # padding defaults: w0 -> sentinel row 0; others -> shared row 32768
    Koff = np.vstack([np.zeros(NB, np.int64), np.cumsum(Ks, 0)[:-1]])
    base_list = np.zeros(nslots * P, np.int64)
    for b in range(NB):
        for w in range(1, NW):
            s0 = (soff[b] + Koff[w, b]) * P
            s1 = (soff[b] + Koff[w, b] + Ks[w, b]) * P
            base_list[s0:s1] = W - bases[w]
    lists = [base_list.copy() for _ in range(NCORE)]

    key2 = (core * R + row) * NW + win
    srt = np.argsort(key2, kind="stable")
    kk_ = key2[srt]
    grp_start = np.r_[0, np.nonzero(np.diff(kk_))[0] + 1]
    sizes = np.diff(np.r_[grp_start, len(kk_)])
    within = np.arange(len(kk_)) - np.repeat(grp_start, sizes)
    ks_ = np.zeros(len(src), np.int64)
    ks_[srt] = within
    b_of = row // P
    p_of = row % P
    slot = ks_ + Koff[win, b_of]
    lpos = (soff[b_of] + slot) * P + p_of
    for c in range(NCORE):
        m = core == c
        lists[c][lpos[m]] = li[m]

    idx_w = [wrap_idx(lists[c]) for c in range(NCORE)]

    # ---- phase-1 inputs
    W1p = W1 * ln_g[:, None]
    W1pad = np.zeros((cfg.d_in_pad, cfg.d_hid), np.float32)
    W1pad[:cfg.d_in] = W1p
    cvec_flat = b1 + ln_b @ W1
    cvec = cvec_flat.reshape(cfg.d_hid // P, P).T.astype(np.float32).copy()
    b2t = b2.reshape(cfg.d_out // P, P).T.astype(np.float32).copy()
    onep = np.zeros((8, P), np.float32)
    onep[0] = 1.0
    ones1 = np.ones((P, 1), np.float32)

    att_src_e = np.zeros((cfg.d_head, H), np.float32)
    att_dst_e = np.zeros((cfg.d_head, H), np.float32)
    for h in range(H):
        att_src_e[h * C:(h + 1) * C, h] = att_src[h]
        att_dst_e[h * C:(h + 1) * C, h] = att_dst[h]
    # c-major column permutation for the g table: col c*H+h <- h*C+c
    cm_cols = np.empty(cfg.d_head, np.int64)
    for c in range(C):
        for h in range(H):
            cm_cols[c * H + h] = h * C + c
    Wg_cm = Wg[:, cm_cols]
    Wgp = np.concatenate([Wg_cm, Wg @ att_src_e], axis=1).astype(np.float32)
    Wrp = np.concatenate([Wr + 0.0, Wg @ att_dst_e], axis=1).astype(np.float32)

    xts = []
    for c in range(NCORE):
        ids = core_nodes[c]
        xs = np.zeros((R, cfg.d_in), np.float32)
        msk = ids >= 0
        xs[msk] = x[ids[msk]]
        xt = np.zeros((cfg.d_in_pad, R), np.float32)
        xt[:cfg.d_in] = xs.T
        xts.append(xt.astype(NP_BF16))

    bg_cm = bg.reshape(H, C).T.flatten()
    bg_b = np.tile(bg_cm, (P, 1)).astype(NP_BF16)
    W3 = cfg.d_head + cfg.H
    brpad_t = np.zeros((P, W3), np.float32)
    brpad_t[:, :cfg.d_head] = np.tile(br.astype(np.float32), (P, 1))

    meta = dict(core_nodes=core_nodes, pos=pos, trow=trow,
                Ks=[list(map(int, Ks[w])) for w in range(NW)],
                idx=idx_w, bg_b=bg_b)
    p1_shared = dict(
        W1p=W1pad.astype(NP_BF16), W2=W2.astype(NP_BF16),
        Wgp=Wgp.astype(NP_BF16), Wrp=Wrp.astype(NP_BF16),
        onep=onep.astype(NP_BF16),
        ones1=ones1.astype(NP_BF16), cvec=cvec, b2v=b2t, brpad=brpad_t)
    p1_maps = [dict(xT=xts[c], **p1_shared) for c in range(NCORE)]
    return p1_maps, meta


def make_sentinel_row(cfg: Cfg) -> np.ndarray:
    row = np.zeros(cfg.row_w, NP_BF16)
    row[cfg.d_head:cfg.d_head + cfg.H] = NP_BF16(-200.0)
    return row


def build_p2_maps(cfg: Cfg, meta, gtabs, ress, adsts):
    cat = np.concatenate(gtabs, axis=0)        # [TR, 384] bf16, position order
    gtab_full = np.empty_like(cat)
    gtab_full[meta["trow"]] = cat              # permute to table-row order
    sent = make_sentinel_row(cfg)
    gtab_full[0] = sent
    gtab_full[cfg.window] = sent
    p2_maps = []
    for c in range(cfg.n_cores):
        ad = adsts[c]  # [R, H] f32, position order
        adt = ad.reshape(cfg.n_batches, P, cfg.H).transpose(1, 0, 2)
        p2_maps.append(dict(
            gtab=gtab_full, idx=meta["idx"][c],
            adt=adt.astype(NP_BF16).copy(),
            resi=ress[c], bgb=meta["bg_b"],
        ))
    return p2_maps


def kernel(**inputs) -> np.ndarray:
    cfg = CFG
    N = cfg.n_nodes
    NCORE = cfg.n_cores
    DH = cfg.d_head

    p1_maps, meta = prep(cfg, **inputs)

    nc1 = build_phase1(cfg)
    r1 = run_bass_kernel_spmd(nc1, p1_maps, core_ids=list(range(NCORE)))
    gtabs = [r1.results[c]["gtab"] for c in range(NCORE)]
    ress = [r1.results[c]["res"] for c in range(NCORE)]
    adsts = [r1.results[c]["adst"] for c in range(NCORE)]

    nc2 = build_phase2(cfg, meta["Ks"])
    p2_maps = build_p2_maps(cfg, meta, gtabs, ress, adsts)
    r2 = run_bass_kernel_spmd(nc2, p2_maps, core_ids=list(range(NCORE)))

    out = np.zeros((N, DH), np.float32)
    for c in range(NCORE):
        ids = meta["core_nodes"][c]
        msk = ids >= 0
        out[ids[msk]] = r2.results[c]["outp"][msk]
    return out
